# revision 98
# baseline (speedup 1.0000x reference)
# Trainium2 Bass kernel for nn_DecoderBlock (B=4, T=2048, E=1024, H=16, D=64, FF=4096).
#
# Sharding: 8-way data parallel, zero collectives. Core c = 2*b + h handles batch b
# and the interleaved half of the sequence: 128-row q-blocks {2s+h : s=0..7}
# (1024 q rows per core). K/V are computed per-core for the full T=2048 rows of its
# batch (duplicated across the two cores of a batch pair) so attention needs no
# cross-core communication. The interleaved block assignment makes the causal
# work pattern identical on every core (uniform SPMD program): q-slot s statically
# attends keys [0, 256*(s+1)), with a per-core {0,1} multiplicative mask (input
# data) handling the parity-dependent diagonal.
#
# On-chip layout: activations flow feature-major (S^T = [k, q]) through attention so
# softmax needs no transposes of the probability matrix. Softmax uses no max
# subtraction (scores are ~N(0, 0.25^2) by construction); 1/denominator is
# exp(-ln(d)) so the whole kernel uses one ACT table set (exp/ln/relu).
# fp8e4m3 DoubleRow matmuls (two 128-deep k-planes per instruction, 0.5 PE
# cycles/row = 4x bf16):
#  - Q/K projections: single-chain fp8 (softmax absorbs score quantization,
#    measured numerically free). rhs is an fp8 shadow of the LN1 output
#    (lnf8), copied from bf16 by the otherwise-idle GPSIMD engine.
#  - V / FFN1 / FFN2: double-split fp8, out = a@u + a@v + b@u with
#    a=fp8(act), b=fp8(act-a), u=fp8(w*s), v=fp8(w*s-u): 3 DR chains = 0.75x
#    bf16 PE time at ~bf16 accuracy (pure fp8 on any of these costs ~1.5e-2
#    max-rel against the 2e-2 gate; the split costs ~1e-3). The b-chain is
#    ordered last so its residual lands off the critical path.
#  - proj + attention scores/AV stay bf16.
# All descales are free by construction: weight scales are powers of two
# (x32, x64 for w2); wv's x32 makes vS carry 32*v and the ones-block is
# memset to 32.0 so the softmax denominator cancels it in do_norm; proj_w
# and xqp are host-scaled x64 so the whole post-attention residual stream
# runs x64 on-chip (LN2 is scale-invariant) and the host divides the final
# output by 64 after the gather; the FFN1 descale rides the relu's scale
# immediate. Softmax denominators invert on DVE via reciprocal_approx_fast
# (full-tile: custom DVE ops require partition base 0 — base-64 slices
# mis-execute on hardware). LN2 stats+apply run inside the attention tail on
# DVE/GPSIMD; only transposes + fp8 split stores remain in the FFN bridge.
# Matmuls accumulate in fp32 PSUM; LN statistics and the final output are
# fp32. LN gains (g1,g2) and the attention 1/sqrt(E) scale are folded into
# the weights on the host; beta terms become per-feature biases.

import numpy as np
import ml_dtypes
from contextlib import ExitStack

BF16 = ml_dtypes.bfloat16
F8 = ml_dtypes.float8_e4m3

B, T, E, H, D, FF = 4, 2048, 1024, 16, 64, 4096
M = 1024          # q rows per core
NCORES = 8
NS = 8            # q slots (128 rows) per core
ET = E // 128     # 8 e-tiles
TK = T // 128     # 16 k-tiles
FT = FF // 128    # 32 ff-tiles
NP = H // 2       # 8 head pairs
EPS = 1e-5

_CACHE = {}


def _build(repeat=1):
    """Build (and cache) the Bass module for one core's uniform program.

    repeat>1 emits the whole body N times (identical I/O) — used only for
    slope-based wall-clock timing of one body on hardware.
    """
    key = ("nc", repeat)
    if key in _CACHE:
        return _CACHE[key]

    import concourse.bacc as bacc
    import concourse.tile as tile
    import concourse.mybir as mybir
    from concourse import masks as cmasks

    dt = mybir.dt
    f32, bf16, f8 = dt.float32, dt.bfloat16, dt.float8e4
    AF = mybir.ActivationFunctionType
    OP = mybir.AluOpType
    DR = mybir.MatmulPerfMode.DoubleRow

    nc = bacc.Bacc("TRN2", target_bir_lowering=False, debug=False,
                   num_devices=NCORES)

    # Every activation we use (Exp, Ln, Relu, Copy, Identity) lives in the
    # 'natural_log_exp_and_others' table set. The default per-function set
    # choice alternates home sets (exp_and_others vs natural_log), inserting
    # ~80 ACT table loads (~100us). Restrict the chooser to the one set that
    # covers everything -> a single load.
    import types
    import bass_rust as _br

    def _insert_act_loads_one_set(self):
        has_activation = any(
            isinstance(i, mybir.InstActivation)
            for b in self.main_func.blocks for i in b.instructions)
        if not has_activation:
            return
        tabs = bacc.get_activation_tables(self.m.arch)
        ours = {mybir.ActivationFunctionType.Exp, mybir.ActivationFunctionType.Ln,
                mybir.ActivationFunctionType.Relu, mybir.ActivationFunctionType.Copy,
                mybir.ActivationFunctionType.Identity}
        filt = []
        for name, fns in tabs.items():
            if name == "natural_log_exp_and_others":
                assert ours <= fns
                filt.append((name, fns))
            else:
                filt.append((name, fns - ours))
        _br.insert_act_table_loads(self, filt)

    nc.insert_act_table_loads = types.MethodType(_insert_act_loads_one_set, nc)

    # ----- DRAM I/O -----
    x_full = nc.dram_tensor("x_full", [T, E], bf16, kind="ExternalInput").ap()
    xqp = nc.dram_tensor("xqp", [M, E], bf16, kind="ExternalInput").ap()
    # weights arrive pre-arranged on the host into SBUF layout
    # [128 partitions, <tile dims>] so each DMA is one long contiguous run
    # per partition (minimal descriptor count).
    wq = nc.dram_tensor("wq", [128, ET * H * D], f8, kind="ExternalInput").ap()
    wk = nc.dram_tensor("wk", [128, ET * H * D], f8, kind="ExternalInput").ap()
    wvu = nc.dram_tensor("wvu", [128, ET * H * D], f8, kind="ExternalInput").ap()
    wvv = nc.dram_tensor("wvv", [128, ET * H * D], f8, kind="ExternalInput").ap()
    projw = nc.dram_tensor("projw", [128, NP * E], bf16, kind="ExternalInput").ap()
    w1u = nc.dram_tensor("w1u", [128, FT * ET * 128], f8,
                         kind="ExternalInput").ap()
    w1v = nc.dram_tensor("w1v", [128, FT * ET * 128], f8,
                         kind="ExternalInput").ap()
    w2u = nc.dram_tensor("w2u", [128, FT * E], f8, kind="ExternalInput").ap()
    w2v = nc.dram_tensor("w2v", [128, FT * E], f8, kind="ExternalInput").ap()
    qb_d = nc.dram_tensor("qb", [128, ET], f32, kind="ExternalInput").ap()
    kb_d = nc.dram_tensor("kb", [128, ET], f32, kind="ExternalInput").ap()
    vb_d = nc.dram_tensor("vb", [128, H * D], bf16, kind="ExternalInput").ap()
    b1_d = nc.dram_tensor("b1", [128, FT], f32, kind="ExternalInput").ap()
    bf2_d = nc.dram_tensor("bf2b", [128, E], bf16, kind="ExternalInput").ap()
    maskE_d = nc.dram_tensor("maskE", [128, 256], bf16, kind="ExternalInput").ap()
    maskO_d = nc.dram_tensor("maskO", [128, 256], bf16, kind="ExternalInput").ap()
    out = nc.dram_tensor("out", [M, E], f32, kind="ExternalOutput").ap()

    with tile.TileContext(nc) as tc:
      for _rep in range(repeat):
        es = ExitStack()
        with es:
            # ---------- constants (whole kernel) ----------
            constp = es.enter_context(tc.tile_pool(name="const", bufs=1))
            ident = constp.tile([128, 128], bf16)
            cmasks.make_identity(nc, ident[:])
            maskE = constp.tile([128, 256], bf16)
            nc.sync.dma_start(maskE[:], maskE_d)
            maskO = constp.tile([128, 256], bf16)
            nc.sync.dma_start(maskO[:], maskO_d)
            qb = constp.tile([128, ET], f32)
            nc.sync.dma_start(qb[:], qb_d)
            kb = constp.tile([128, ET], f32)
            nc.sync.dma_start(kb[:], kb_d)
            vb = constp.tile([128, H * D], bf16)
            nc.sync.dma_start(vb[:], vb_d)
            b1 = constp.tile([128, FT], f32)
            nc.sync.dma_start(b1[:], b1_d)
            bf2 = constp.tile([128, E], bf16)
            nc.sync.dma_start(bf2[:], bf2_d)
            eps_t = constp.tile([128, 1], f32)
            nc.gpsimd.memset(eps_t[:], EPS)

            # helper: layernorm one 128-row chunk (fp32 src slice in SBUF) and
            # write the transposed bf16 result into dst_T[:, et, col:col+128].
            def ln_stats(src, statp):
                st = statp.tile([128, 2, 6], f32, tag="st")
                for g in range(2):
                    nc.vector.bn_stats(st[:, g, :], src[:, g * 512:(g + 1) * 512])
                ag = statp.tile([128, 2], f32, tag="ag")
                nc.vector.bn_aggr(ag[:], st[:])
                lv = statp.tile([128, 1], f32, tag="lv")
                nc.scalar.activation(lv[:], ag[:, 1:2], AF.Ln, bias=eps_t[:])
                rstd = statp.tile([128, 1], f32, tag="rstd")
                nc.scalar.activation(rstd[:], lv[:], AF.Exp, scale=-0.5)
                return ag, rstd

            def ln_apply(lc, src, ag, rstd, statp, use_act):
                # normalize-apply, split across two engines. use_act=True
                # (LN1 phase: ACT idle, DVE loaded): ACT Identity computes
                # (src*rstd - m*rstd) for half0, GPSIMD half1. use_act=False
                # (LN2, during attention: ACT saturated): DVE + GPSIMD.
                if use_act:
                    mr = statp.tile([128, 1], f32, tag="mr")
                    nc.vector.tensor_scalar(mr[:], ag[:, 0:1], rstd[:], -1.0,
                                            OP.mult, OP.mult)
                    nc.scalar.activation(lc[:, 0:512], src[:, 0:512],
                                         AF.Identity, bias=mr[:],
                                         scale=rstd[:])
                else:
                    nc.vector.tensor_scalar(lc[:, 0:512], src[:, 0:512],
                                            ag[:, 0:1], rstd[:],
                                            OP.subtract, OP.mult)
                nc.gpsimd.tensor_scalar(lc[:, 512:1024], src[:, 512:1024],
                                        ag[:, 0:1], rstd[:],
                                        OP.subtract, OP.mult)

            def ln_chunk(src, dst_T, col, statp, lnstage, tpsum, ci,
                         stats=None, dst8=None, col8=0, copies_act=True,
                         lc=None):
                if lc is None:
                    ag, rstd = (stats if stats is not None
                                else ln_stats(src, statp))
                    lc = lnstage.tile([128, E], bf16)
                    ln_apply(lc, src, ag, rstd, statp, copies_act)
                for et in range(ET):
                    tp = tpsum.tile([128, 128], bf16)
                    nc.tensor.transpose(tp[:],
                                        lc[:, et * 128:(et + 1) * 128],
                                        ident[:])
                    dst = dst_T[:, et, col:col + 128]
                    if copies_act and (et + ci) % 2 == 0:
                        nc.scalar.copy(dst, tp[:])
                    else:
                        nc.vector.tensor_copy(dst, tp[:])
                    # (copies stay off GPSIMD: it cannot read PSUM)
                if dst8 is not None:
                    # fp8 shadow for the Q/K DoubleRow rhs, off the busy
                    # engines: GPSIMD reads the finished bf16 strip. Two
                    # halves so downstream chains start at half-done.
                    nc.gpsimd.tensor_copy(dst8[:, 0:4, col8:col8 + 128],
                                          dst_T[:, 0:4, col:col + 128])
                    nc.gpsimd.tensor_copy(dst8[:, 4:8, col8:col8 + 128],
                                          dst_T[:, 4:8, col:col + 128])

            # ---------- scope B: qT/kT/v (strict stack nesting) ----------
            xmid = es.enter_context(tc.tile_pool(name="xmidp", bufs=1)).tile(
                [128, NS, E], bf16)
            # LN2 stats+apply run during the attention tail (DVE/GPSIMD);
            # only the transposes+copies stay in scope D. lc2 holds the
            # normalized (pre-transpose) chunks across the scope boundary.
            lc2 = es.enter_context(tc.tile_pool(name="ln2s", bufs=1)).tile(
                [128, NS, E], bf16)
            statp2o = es.enter_context(tc.tile_pool(name="statp2o", bufs=6))
            with ExitStack() as sB:
                qT = sB.enter_context(tc.tile_pool(name="qTp", bufs=1)).tile(
                    [128, NP, M], bf16)
                kT = sB.enter_context(tc.tile_pool(name="kTp", bufs=1)).tile(
                    [128, NP, T], bf16)
                # per-pair AV stationary operand [V_h0 | ones(64) | V_h1]:
                # head0 reads cols 0:128, head1 cols 64:192 — the shared ones
                # block makes the same matmul that accumulates attn@V also
                # accumulate the softmax denominator (replicated on the 64
                # out-rows opposite each head's data rows).
                # vS carries 32*v (fp8 wv split is stored x32); the ones
                # block becomes 32.0 so the denominator picks up the same
                # scale and do_norm's divide cancels it exactly.
                vS = sB.enter_context(tc.tile_pool(name="vp", bufs=1)).tile(
                    [128, TK, NP, 192], bf16)

                # ---------- scope A: LN1 + QKV projections ----------
                with ExitStack() as sA:
                    wpool = sA.enter_context(tc.tile_pool(name="wpool", bufs=1))
                    stage = sA.enter_context(tc.tile_pool(name="xstage", bufs=4))
                    lnstage = sA.enter_context(tc.tile_pool(name="lnstage", bufs=2))
                    statp = sA.enter_context(tc.tile_pool(name="statp", bufs=6))
                    tpsum = sA.enter_context(
                        tc.tile_pool(name="tpsum", bufs=4, space="PSUM"))
                    qps = sA.enter_context(
                        tc.tile_pool(name="qps", bufs=2, space="PSUM"))

                    # lnf chunks rotate through a 3-deep stage: V consumes
                    # each chunk as soon as it lands; Q/K read the persistent
                    # fp8 shadow lnf8.
                    lnfp = sA.enter_context(tc.tile_pool(name="lnfp", bufs=4))
                    lnf8 = sA.enter_context(tc.tile_pool(name="lnf8p", bufs=1)).tile(
                        [128, ET, T], f8)
                    # LN1 over x_full (host-permuted: q rows are chunks 0..7,
                    # the pair-core's rows are chunks 8..15) -> lnf. Q^T, each
                    # K^T token-quarter and each V chunk are emitted as soon
                    # as the lnf region they read is complete, so the PE
                    # works through projections while the LN chains run.
                    wq_sb = wpool.tile([128, ET, H * D], f8, tag="wq")
                    wk_sb = wpool.tile([128, ET, H * D], f8, tag="wk")
                    wvu_sb = wpool.tile([128, ET, H * D], f8, tag="wvu")
                    wvv_sb = wpool.tile([128, ET, H * D], f8, tag="wvv")
                    nc.sync.dma_start(wvu_sb[:], wvu.rearrange(
                        "p (et n) -> p et n", et=ET))
                    nc.sync.dma_start(wvv_sb[:], wvv.rearrange(
                        "p (et n) -> p et n", et=ET))
                    lnbp = sA.enter_context(tc.tile_pool(name="lnbp", bufs=3))

                    def v_chunk(t, lnfc):
                        # split-fp8: v = a@u + b@u + a@v with a=fp8(lnf),
                        # b=fp8(lnf-a), u/v the fp8 split of 32*wv. 12 DR
                        # matmuls replace 16 bf16 ones (0.75x PE).
                        a8 = lnf8[:, :, t * 128:(t + 1) * 128]
                        b8 = lnbp.tile([128, ET, 128], f8, tag="lnb")
                        nc.vector.tensor_tensor(b8[:, 0:4], lnfc[:, 0:4],
                                                a8[:, 0:4], OP.subtract)
                        nc.gpsimd.tensor_tensor(b8[:, 4:8], lnfc[:, 4:8],
                                                a8[:, 4:8], OP.subtract)
                        ps = qps.tile([128, 1024], f32)
                        for hc in range(2):
                            cols = slice(hc * 512, (hc + 1) * 512)
                            chains = [(a8, wvu_sb), (a8, wvv_sb),
                                      (b8, wvu_sb)]
                            for ci, (act, wt) in enumerate(chains):
                                for ep in range(ET // 2):
                                    nc.tensor.matmul(
                                        ps[:, cols],
                                        lhsT=act[:, 2 * ep:2 * ep + 2, :],
                                        rhs=wt[:, 2 * ep:2 * ep + 2, cols],
                                        start=(ci == 0 and ep == 0),
                                        stop=(ci == 2 and ep == ET // 2 - 1),
                                        perf_mode=DR)
                        psv = ps[:].rearrange("p (np h d) -> p np h d", np=NP,
                                              h=2)
                        vbv = vb[:].rearrange("p (np h d) -> p np h d", np=NP,
                                              h=2)
                        nc.vector.tensor_add(
                            vS[:, t, :, 0:64], psv[:, :, 0, :], vbv[:, :, 0, :])
                        nc.vector.tensor_add(
                            vS[:, t, :, 128:192], psv[:, :, 1, :],
                            vbv[:, :, 1, :])

                    def k_quarter(tq, ms):
                        for m in ms:
                            ps = qps.tile([128, 512], f32)
                            for ep in range(ET // 2):
                                nc.tensor.matmul(
                                    ps[:],
                                    lhsT=wk_sb[:, 2 * ep:2 * ep + 2,
                                               m * 128:(m + 1) * 128],
                                    rhs=lnf8[:, 2 * ep:2 * ep + 2,
                                             tq * 512:(tq + 1) * 512],
                                    start=(ep == 0), stop=(ep == ET // 2 - 1),
                                    perf_mode=DR)
                            # descale+bias on ACT (idle in this phase)
                            nc.scalar.activation(
                                kT[:, m, tq * 512:(tq + 1) * 512], ps[:],
                                AF.Identity, bias=kb[:, m:m + 1],
                                scale=1.0 / 32)

                    def q_proj(ms):
                        for m in ms:
                            ps = qps.tile([128, 1024], f32)
                            for qc in range(2):
                                for ep in range(ET // 2):
                                    nc.tensor.matmul(
                                        ps[:, qc * 512:(qc + 1) * 512],
                                        lhsT=wq_sb[:, 2 * ep:2 * ep + 2,
                                                   m * 128:(m + 1) * 128],
                                        rhs=lnf8[:, 2 * ep:2 * ep + 2,
                                                 qc * 512:(qc + 1) * 512],
                                        start=(ep == 0),
                                        stop=(ep == ET // 2 - 1),
                                        perf_mode=DR)
                            nc.scalar.activation(
                                qT[:, m, :], ps[:], AF.Identity,
                                bias=qb[:, m:m + 1],
                                scale=float(E) ** -0.5 / 32)

                    for c in range(TK):
                        xc = stage.tile([128, E], bf16)
                        nc.scalar.dma_start(
                            xc[:], x_full[c * 128:(c + 1) * 128, :])
                        lnfc = lnfp.tile([128, ET, 128], bf16, tag="lnfc")
                        ln_chunk(xc[:], lnfc, 0, statp, lnstage,
                                 tpsum, c, dst8=lnf8, col8=c * 128)
                        if c == 0:
                            nc.sync.dma_start(wk_sb[:], wk.rearrange(
                                "p (et n) -> p et n", et=ET))
                        if c == 1:
                            nc.sync.dma_start(wq_sb[:], wq.rearrange(
                                "p (et n) -> p et n", et=ET))
                        v_chunk(c, lnfc)
                        if c % 4 == 3:
                            k_quarter(c // 4, range(0, 4))
                        if c % 4 == 0 and c > 0:
                            k_quarter(c // 4 - 1, range(4, ET))
                        if c == TK - 1:
                            k_quarter(3, range(4, ET))
                        if c == NS - 1:
                            q_proj(range(0, 4))
                        if c == NS + 1:
                            q_proj(range(4, ET))

                # ---------- attention + output projection ----------
                # half-outer: all pairs finish q-cols [0,512) first; the
                # output projection for q-chunks 0..3 is then dripped one
                # chunk at a time between second-half pairs so the PE works
                # through proj while ACT drains the exp/normalize backlog.
                with ExitStack() as sC:
                    oT = sC.enter_context(tc.tile_pool(name="oTp", bufs=1)).tile(
                        [128, NP, M], bf16)
                    ptp = sC.enter_context(tc.tile_pool(name="ptp", bufs=5))
                    normp = sC.enter_context(tc.tile_pool(name="normp", bufs=2))
                    xqpp = sC.enter_context(tc.tile_pool(name="xqpp", bufs=1))
                    pwp = sC.enter_context(tc.tile_pool(name="pwp", bufs=1))
                    apsum = sC.enter_context(
                        tc.tile_pool(name="apsum", bufs=2, space="PSUM"))
                    spsum = sC.enter_context(
                        tc.tile_pool(name="spsum", bufs=2, space="PSUM"))

                    maskEv = maskE[:].rearrange("p (h q) -> p h q", h=2)
                    maskOv = maskO[:].rearrange("p (h q) -> p h q", h=2)

                    pw_sb = pwp.tile([128, NP, E], bf16)
                    nc.sync.dma_start(pw_sb[:], projw.rearrange(
                        "p (m e) -> p m e", m=NP))
                    nc.gpsimd.memset(vS[:, 0:8, :, 64:128], 32.0)
                    nc.gpsimd.memset(vS[:, 8:16, :, 64:128], 32.0)
                    xq_t = {}

                    def do_norm(av, p, half):
                        # den sits on the 64 rows opposite each head's data;
                        # 1/den via the fast DVE Newton-Raphson reciprocal
                        # (dens are sums of exps, well inside its safe range)
                        # keeps the whole normalize off the exp-saturated ACT.
                        # one full-tile fast reciprocal (custom DVE ops
                        # require partition base 0; the data-row lanes are
                        # junk and never read)
                        rcp = normp.tile([128, 2, 512], f32, tag="rcp")
                        nc.vector.reciprocal_approx_fast(rcp[:], av[:])
                        colr = slice(512 * half, 512 * (half + 1))
                        for h in range(2):
                            dn = slice(64 * (1 - h), 64 * (1 - h) + 64)
                            nc.vector.tensor_mul(
                                oT[64 * h:64 * h + 64, p, colr],
                                av[64 * h:64 * h + 64, h, :],
                                rcp[dn, h, :])

                    # permuted key order: chunks 0..7 are this core's parity
                    # (incl. the causal diagonal), 8..15 the pair-core's
                    # (strictly past or future, selected by the all-ones/
                    # all-zero parity mask).
                    CHUNKS = [
                        [(kt, 128 * kt, "tri") for kt in range(4)] +
                        [(kt, 128 * (kt - 8), "par") for kt in range(8, 12)],
                        [(kt, 0, None) for kt in range(4)] +
                        [(kt, 128 * (kt - 4), "tri") for kt in range(4, 8)] +
                        [(kt, 0, None) for kt in range(8, 12)] +
                        [(kt, 128 * (kt - 12), "par") for kt in range(12, 16)],
                    ]

                    def do_av(item):
                        av, p, half, kt, qlo, sp, pt = item
                        for h in range(2):
                            nc.tensor.matmul(
                                av[:, h, qlo:512],
                                lhsT=vS[:, kt, p, 64 * h:64 * h + 128],
                                rhs=pt[:, h, qlo:512],
                                start=(kt == 0), stop=sp,
                                skip_group_check=True)
                        if sp:
                            do_norm(av, p, half)

                    def proj_qm(qm):
                        if qm // 4 not in xq_t:
                            xqh = xqpp.tile([128, 4, E], bf16, tag="xq")
                            xq_t[qm // 4] = xqh
                        xqh = xq_t[qm // 4]
                        nc.sync.dma_start(
                            xqh[:, qm % 4, :], xqp[qm * 128:(qm + 1) * 128, :])
                        ps = apsum.tile([128, 1024], f32, tag="av")
                        for ec in range(2):
                            for pk in range(NP):
                                nc.tensor.matmul(
                                    ps[:, ec * 512:(ec + 1) * 512],
                                    lhsT=oT[:, pk, qm * 128:(qm + 1) * 128],
                                    rhs=pw_sb[:, pk, ec * 512:(ec + 1) * 512],
                                    start=(pk == 0), stop=(pk == NP - 1))
                        nc.vector.tensor_add(
                            xmid[:, qm, :], ps[:], xqh[:, qm % 4, :])

                    def ln2_sa(qm):
                        ag, rstd = ln_stats(xmid[:, qm, :], statp2o)
                        ln_apply(lc2[:, qm, :], xmid[:, qm, :], ag, rstd,
                                 statp2o, False)
                        # after LN2 consumed xmid, fold the final bf2 bias in
                        nc.gpsimd.tensor_add(xmid[:, qm, :], xmid[:, qm, :],
                                             bf2[:])

                    def attn_half(half, interleave=()):
                        chunks = CHUNKS[half]
                        last_kt = chunks[-1][0]
                        pend = []
                        for p in range(NP):
                            av = apsum.tile([128, 2, 512], f32, tag="av")
                            for kt, qlo, mk in chunks:
                                ps = spsum.tile([128, 2, 512], f32)
                                for h in range(2):
                                    nc.tensor.matmul(
                                        ps[:, h, qlo:512],
                                        lhsT=kT[64 * h:64 * h + 64, p,
                                                kt * 128:(kt + 1) * 128],
                                        rhs=qT[64 * h:64 * h + 64, p,
                                               512 * half + qlo:
                                               512 * (half + 1)],
                                        start=True, stop=True)
                                pt = ptp.tile([128, 2, 512], bf16)
                                nc.scalar.activation(
                                    pt[:, :, qlo:512], ps[:, :, qlo:512],
                                    AF.Exp)
                                if mk is not None:
                                    mkv = maskEv if mk == "tri" else maskOv
                                    nc.vector.tensor_mul(
                                        pt[:, :, qlo:qlo + 128],
                                        pt[:, :, qlo:qlo + 128], mkv)
                                pend.append((av, p, half, kt, qlo,
                                             kt == last_kt, pt))
                                if len(pend) > 3:
                                    do_av(pend.pop(0))
                            if p % 2 == 1 and p // 2 < len(interleave):
                                while pend:
                                    do_av(pend.pop(0))
                                interleave[p // 2]()
                        while pend:
                            do_av(pend.pop(0))

                    attn_half(0)
                    attn_half(1, interleave=[
                        (lambda qm=qm: (proj_qm(qm), ln2_sa(qm)))
                        for qm in range(4)])
                    for qm in range(4, NS):
                        proj_qm(qm)
                        ln2_sa(qm)

            # ---------- scope D: LN2 + FFN ----------
            # LN2 q-chunks 0..3 interleave with the tail projections; FFN1's
            # first-half fm groups interleave with LN2 chunks 4..7 so the
            # PE never drains while DVE/ACT run the layernorm chains.
            with ExitStack() as sD:
                ln2p = sD.enter_context(tc.tile_pool(name="ln2p", bufs=1))
                ln2a = ln2p.tile([128, ET, M], f8, tag="a")
                ln2b = ln2p.tile([128, ET, M], f8, tag="b")
                w2p = sD.enter_context(tc.tile_pool(name="w2p", bufs=1))
                w2u_sb = w2p.tile([128, FT, E], f8, tag="u")
                w2v_sb = w2p.tile([128, FT, E], f8, tag="v")

                tpsum2 = sD.enter_context(
                    tc.tile_pool(name="tpsum2", bufs=2, space="PSUM"))
                rtp = sD.enter_context(tc.tile_pool(name="rtp", bufs=1))
                rbfp = sD.enter_context(tc.tile_pool(name="rbfp", bufs=3))
                w1p = sD.enter_context(tc.tile_pool(name="w1p", bufs=6))
                zps = sD.enter_context(
                    tc.tile_pool(name="zps", bufs=2, space="PSUM"))
                ops = sD.enter_context(
                    tc.tile_pool(name="ops", bufs=2, space="PSUM"))
                outp = sD.enter_context(tc.tile_pool(name="outp", bufs=3))

                def ln2_qm(qm):
                    # stats+apply already ran in the attention tail. Here:
                    # transpose (PE), then the fp8 split stores — a on ACT,
                    # b = tp - a on DVE.
                    col = qm * 128
                    lc = lc2[:, qm, :]
                    for et in range(ET):
                        tp = tpsum2.tile([128, 128], bf16)
                        nc.tensor.transpose(tp[:],
                                            lc[:, et * 128:(et + 1) * 128],
                                            ident[:])
                        nc.scalar.copy(ln2a[:, et, col:col + 128], tp[:])
                        nc.vector.tensor_tensor(
                            ln2b[:, et, col:col + 128], tp[:],
                            ln2a[:, et, col:col + 128], OP.subtract)

                rT_tiles = {}

                def ffn1_fm(half, fm):
                    if half not in rT_tiles:
                        ra = rtp.tile([128, FT, 512], f8, tag="rTa")
                        rb = rtp.tile([128, FT, 512], f8, tag="rTb")
                        rT_tiles[half] = (ra, rb)
                    ra, rb = rT_tiles[half]
                    w1f = w1p.tile([128, 2, ET, 128], f8)
                    nc.sync.dma_start(
                        w1f[:, 0], w1u.rearrange("p (fm et f) -> p fm et f",
                                                 fm=FT, et=ET)[:, fm])
                    nc.sync.dma_start(
                        w1f[:, 1], w1v.rearrange("p (fm et f) -> p fm et f",
                                                 fm=FT, et=ET)[:, fm])
                    zp = zps.tile([128, 512], f32)
                    cols = slice(half * 512, (half + 1) * 512)
                    chains = [(ln2a, 0), (ln2a, 1), (ln2b, 0)]
                    for ci, (act, wi) in enumerate(chains):
                        for ep in range(ET // 2):
                            nc.tensor.matmul(
                                zp[:],
                                lhsT=w1f[:, wi, 2 * ep:2 * ep + 2, :],
                                rhs=act[:, 2 * ep:2 * ep + 2, cols],
                                start=(ci == 0 and ep == 0),
                                stop=(ci == 2 and ep == ET // 2 - 1),
                                perf_mode=DR)
                    # relu (scale descales the x32 of w1), then fp8 split:
                    # a copy on ACT, b = r - a on DVE.
                    # rbf = relu(z + 32*b1) = 32*relu(z/32+b1); the x32
                    # rides the residual stream (host-descaled x2048).
                    rbf = rbfp.tile([128, 512], bf16)
                    nc.scalar.activation(rbf[:], zp[:], AF.Relu,
                                         bias=b1[:, fm:fm + 1])
                    # fp8 split: a on the idle GPSIMD (SBUF->SBUF), b on DVE
                    nc.gpsimd.tensor_copy(ra[:, fm, :], rbf[:])
                    nc.vector.tensor_tensor(rb[:, fm, :], rbf[:],
                                            ra[:, fm, :], OP.subtract)

                def ffn2_qq(half, qq):
                    ra, rb = rT_tiles[half]
                    qm = half * 4 + qq
                    ot = outp.tile([128, E], f32)
                    op = ops.tile([128, 1024], f32)
                    for ec in range(2):
                        cols = slice(ec * 512, (ec + 1) * 512)
                        chains = [(ra, w2u_sb), (ra, w2v_sb), (rb, w2u_sb)]
                        for ci, (act, wt) in enumerate(chains):
                            for fp_ in range(FT // 2):
                                nc.tensor.matmul(
                                    op[:, cols],
                                    lhsT=act[:, 2 * fp_:2 * fp_ + 2,
                                             qq * 128:(qq + 1) * 128],
                                    rhs=wt[:, 2 * fp_:2 * fp_ + 2, cols],
                                    start=(ci == 0 and fp_ == 0),
                                    stop=(ci == 2 and fp_ == FT // 2 - 1),
                                    perf_mode=DR)
                        nc.vector.tensor_add(
                            ot[:, cols], op[:, cols], xmid[:, qm, cols])
                        nc.scalar.dma_start(
                            out[qm * 128:(qm + 1) * 128, cols], ot[:, cols])

                for qm in range(4):
                    ln2_qm(qm)
                # LN2 chunks 4..7 drip between the first 4 fm-groups of
                # FFN1-half0 (which only needs ln2T token cols 0:512).
                for g in range(4):
                    ln2_qm(4 + g)
                    nc.sync.dma_start(
                        w2u_sb[:, g * 8:(g + 1) * 8, :],
                        w2u.rearrange("p (ft e) -> p ft e",
                                      ft=FT)[:, g * 8:(g + 1) * 8, :])
                    nc.sync.dma_start(
                        w2v_sb[:, g * 8:(g + 1) * 8, :],
                        w2v.rearrange("p (ft e) -> p ft e",
                                      ft=FT)[:, g * 8:(g + 1) * 8, :])
                    for fm in range(g * 8, g * 8 + 8):
                        ffn1_fm(0, fm)
                for qq in range(4):
                    ffn2_qq(0, qq)
                rT_tiles.pop(0)
                for fm in range(FT):
                    ffn1_fm(1, fm)
                for qq in range(4):
                    ffn2_qq(1, qq)

    nc.compile()
    _CACHE[key] = nc
    return nc


def _prep_inputs(x, wq, wk, wv, proj_w, proj_b, g1, beta1, g2, beta2, w1, bf1,
                 w2, bf2):
    """Host-side sharding + weight folding. Returns list of 8 in_maps."""
    f32 = np.float32
    x = np.asarray(x, f32)
    scale = float(E) ** -0.5

    Wq = np.asarray(wq, f32).transpose(1, 0, 2).reshape(E, H * D)
    Wk = np.asarray(wk, f32).transpose(1, 0, 2).reshape(E, H * D)
    Wv = np.asarray(wv, f32).transpose(1, 0, 2).reshape(E, H * D)
    g1 = np.asarray(g1, f32)
    beta1 = np.asarray(beta1, f32)
    g2 = np.asarray(g2, f32)
    beta2 = np.asarray(beta2, f32)
    w1 = np.asarray(w1, f32)
    w2 = np.asarray(w2, f32)
    bf1 = np.asarray(bf1, f32)
    bf2 = np.asarray(bf2, f32)
    proj_w = np.asarray(proj_w, f32)
    proj_b = np.asarray(proj_b, f32)

    def sb_layout(w, ntile):
        # [ntile*128, N] -> [128, ntile*N] with per-partition contiguous tiles
        n = w.shape[1]
        return np.ascontiguousarray(
            w.reshape(ntile, 128, n).transpose(1, 0, 2).reshape(128, ntile * n))

    # fp8 weight scaling: x32 (x64 for w2) puts sigma at ~1 inside e4m3's
    # normal range. V/FFN weights are double-split (u = fp8(w*s),
    # v = fp8(w*s - u)) for the 3-chain split matmuls; Q/K use u only.
    def split8(w, s):
        u = (w * s).astype(F8)
        v = ((w * s) - u.astype(f32)).astype(F8)
        return u, v

    wq_b = sb_layout((Wq * g1[:, None] * 32).astype(F8), ET)
    wk_b = sb_layout((Wk * g1[:, None] * 32).astype(F8), ET)
    wvu_n, wvv_n = split8(Wv * g1[:, None], 32)
    wvu_b, wvv_b = sb_layout(wvu_n, ET), sb_layout(wvv_n, ET)
    qbias = (beta1 @ Wq) * scale
    kbias = beta1 @ Wk
    vbias = (beta1 @ Wv) * 32
    w1u_n, w1v_n = split8(w1 * g2[:, None], 32)

    def w1_layout(w):
        return np.ascontiguousarray(
            w.reshape(ET, 128, FT, 128).transpose(1, 2, 0, 3)
            .reshape(128, FT * ET * 128))

    w1u_b, w1v_b = w1_layout(w1u_n), w1_layout(w1v_n)
    b1v = bf1 + beta2 @ w1
    w2u_n, w2v_n = split8(w2, 64)
    w2u_b, w2v_b = sb_layout(w2u_n, FT), sb_layout(w2v_n, FT)
    # the residual stream runs x64 on-chip (w2 split carries it; proj_w and
    # xqp are pre-scaled to match); divided back out on the host.
    projw_b = sb_layout((proj_w * 2048).astype(BF16), NP)

    qb = np.ascontiguousarray(qbias.reshape(ET, 128).T, f32)
    kb = np.ascontiguousarray(kbias.reshape(ET, 128).T, f32)
    vb = np.ascontiguousarray(np.broadcast_to(vbias, (128, H * D))).astype(BF16)
    b1m = np.ascontiguousarray(b1v.reshape(FT, 128).T * 32, f32)
    bf2m = np.ascontiguousarray(
        np.broadcast_to(bf2 * 2048, (128, E))).astype(BF16)

    tri = np.triu(np.ones((128, 128), f32))  # [k_row, q_col]: 1 iff k <= q
    zerosm = np.zeros((128, 128), f32)
    # maskE = causal diagonal (all cores); maskO = parity: the pair-core's
    # diagonal-adjacent chunk is strictly past (odd cores) or future (even).
    mO = {0: zerosm, 1: tri * 0 + 1}
    in_maps = []
    for c in range(NCORES):
        b, hpar = c // 2, c % 2
        xc = x[b].reshape(TK, 128, E)
        xq = np.ascontiguousarray(xc[hpar::2].reshape(M, E), f32)
        xperm = np.ascontiguousarray(
            np.concatenate([xc[hpar::2], xc[1 - hpar::2]], axis=0)
            .reshape(T, E)).astype(BF16)
        in_maps.append({
            "x_full": xperm,
            "xqp": ((xq + proj_b[None, :].astype(f32)) * 2048).astype(BF16),
            "wq": wq_b, "wk": wk_b, "wvu": wvu_b, "wvv": wvv_b,
            "projw": projw_b, "w1u": w1u_b, "w1v": w1v_b,
            "w2u": w2u_b, "w2v": w2v_b,
            "qb": qb, "kb": kb, "vb": vb, "b1": b1m, "bf2b": bf2m,
            "maskE": np.ascontiguousarray(
                np.tile(tri, (1, 2))).astype(BF16),
            "maskO": np.ascontiguousarray(
                np.tile(mO[hpar], (1, 2))).astype(BF16),
        })
    return in_maps


def _run(inputs, trace=False):
    from concourse.bass_utils import run_bass_kernel_spmd
    nc = _build()
    in_maps = _prep_inputs(**inputs)
    res = run_bass_kernel_spmd(nc, in_maps, core_ids=list(range(NCORES)),
                               trace=trace)
    full = np.empty((B, T, E), np.float32)
    for c in range(NCORES):
        b, hpar = c // 2, c % 2
        full[b].reshape(TK, 128, E)[hpar::2] = (
            res.results[c]["out"].reshape(NS, 128, E) * np.float32(1 / 2048))
    return full, res


def kernel(**inputs) -> np.ndarray:
    out, _ = _run(inputs, trace=False)
    return out



# revision 108
# speedup vs baseline: 1.0066x; 1.0066x over previous
# Trainium2 Bass kernel for nn_DecoderBlock (B=4, T=2048, E=1024, H=16, D=64, FF=4096).
#
# Sharding: 8-way data parallel, zero collectives. Core c = 2*b + h handles batch b
# and the interleaved half of the sequence: 128-row q-blocks {2s+h : s=0..7}
# (1024 q rows per core). K/V are computed per-core for the full T=2048 rows of its
# batch (duplicated across the two cores of a batch pair) so attention needs no
# cross-core communication. The interleaved block assignment makes the causal
# work pattern identical on every core (uniform SPMD program): q-slot s statically
# attends keys [0, 256*(s+1)), with a per-core {0,1} multiplicative mask (input
# data) handling the parity-dependent diagonal.
#
# On-chip layout: activations flow feature-major (S^T = [k, q]) through attention so
# softmax needs no transposes of the probability matrix. Softmax uses no max
# subtraction (scores are ~N(0, 0.25^2) by construction); 1/denominator is
# exp(-ln(d)) so the whole kernel uses one ACT table set (exp/ln/relu).
# fp8e4m3 DoubleRow matmuls (two 128-deep k-planes per instruction, 0.5 PE
# cycles/row = 4x bf16):
#  - Q/K projections: single-chain fp8 (softmax absorbs score quantization,
#    measured numerically free). rhs is an fp8 shadow of the LN1 output
#    (lnf8), copied from bf16 by the otherwise-idle GPSIMD engine.
#  - V / FFN1 / FFN2: double-split fp8, out = a@u + a@v + b@u with
#    a=fp8(act), b=fp8(act-a), u=fp8(w*s), v=fp8(w*s-u): 3 DR chains = 0.75x
#    bf16 PE time at ~bf16 accuracy (pure fp8 on any of these costs ~1.5e-2
#    max-rel against the 2e-2 gate; the split costs ~1e-3). The b-chain is
#    ordered last so its residual lands off the critical path.
#  - proj + attention scores/AV stay bf16.
# All descales are free by construction: weight scales are powers of two
# (x32, x64 for w2); wv's x32 makes vS carry 32*v and the ones-block is
# memset to 32.0 so the softmax denominator cancels it in do_norm; proj_w
# and xqp are host-scaled x64 so the whole post-attention residual stream
# runs x64 on-chip (LN2 is scale-invariant) and the host divides the final
# output by 64 after the gather; the FFN1 descale rides the relu's scale
# immediate. Softmax denominators invert on DVE via reciprocal_approx_fast
# (full-tile: custom DVE ops require partition base 0 — base-64 slices
# mis-execute on hardware). LN2 stats+apply run inside the attention tail on
# DVE/GPSIMD; only transposes + fp8 split stores remain in the FFN bridge.
# Matmuls accumulate in fp32 PSUM; LN statistics and the final output are
# fp32. LN gains (g1,g2) and the attention 1/sqrt(E) scale are folded into
# the weights on the host; beta terms become per-feature biases.

import numpy as np
import ml_dtypes
from contextlib import ExitStack

BF16 = ml_dtypes.bfloat16
F8 = ml_dtypes.float8_e4m3

B, T, E, H, D, FF = 4, 2048, 1024, 16, 64, 4096
M = 1024          # q rows per core
NCORES = 8
NS = 8            # q slots (128 rows) per core
ET = E // 128     # 8 e-tiles
TK = T // 128     # 16 k-tiles
FT = FF // 128    # 32 ff-tiles
NP = H // 2       # 8 head pairs
EPS = 1e-5

_CACHE = {}


def _build(repeat=1):
    """Build (and cache) the Bass module for one core's uniform program.

    repeat>1 emits the whole body N times (identical I/O) — used only for
    slope-based wall-clock timing of one body on hardware.
    """
    key = ("nc", repeat)
    if key in _CACHE:
        return _CACHE[key]

    import concourse.bacc as bacc
    import concourse.tile as tile
    import concourse.mybir as mybir
    from concourse import masks as cmasks

    dt = mybir.dt
    f32, bf16, f8 = dt.float32, dt.bfloat16, dt.float8e4
    AF = mybir.ActivationFunctionType
    OP = mybir.AluOpType
    DR = mybir.MatmulPerfMode.DoubleRow

    nc = bacc.Bacc("TRN2", target_bir_lowering=False, debug=False,
                   num_devices=NCORES)

    # Every activation we use (Exp, Ln, Relu, Copy, Identity) lives in the
    # 'natural_log_exp_and_others' table set. The default per-function set
    # choice alternates home sets (exp_and_others vs natural_log), inserting
    # ~80 ACT table loads (~100us). Restrict the chooser to the one set that
    # covers everything -> a single load.
    import types
    import bass_rust as _br

    def _insert_act_loads_one_set(self):
        has_activation = any(
            isinstance(i, mybir.InstActivation)
            for b in self.main_func.blocks for i in b.instructions)
        if not has_activation:
            return
        tabs = bacc.get_activation_tables(self.m.arch)
        ours = {mybir.ActivationFunctionType.Exp, mybir.ActivationFunctionType.Ln,
                mybir.ActivationFunctionType.Relu, mybir.ActivationFunctionType.Copy,
                mybir.ActivationFunctionType.Identity}
        filt = []
        for name, fns in tabs.items():
            if name == "natural_log_exp_and_others":
                assert ours <= fns
                filt.append((name, fns))
            else:
                filt.append((name, fns - ours))
        _br.insert_act_table_loads(self, filt)

    nc.insert_act_table_loads = types.MethodType(_insert_act_loads_one_set, nc)

    # ----- DRAM I/O -----
    x_full = nc.dram_tensor("x_full", [T, E], bf16, kind="ExternalInput").ap()
    xqp = nc.dram_tensor("xqp", [M, E], bf16, kind="ExternalInput").ap()
    # weights arrive pre-arranged on the host into SBUF layout
    # [128 partitions, <tile dims>] so each DMA is one long contiguous run
    # per partition (minimal descriptor count).
    wq = nc.dram_tensor("wq", [128, ET * H * D], f8, kind="ExternalInput").ap()
    wk = nc.dram_tensor("wk", [128, ET * H * D], f8, kind="ExternalInput").ap()
    wvu = nc.dram_tensor("wvu", [128, ET * H * D], f8, kind="ExternalInput").ap()
    wvv = nc.dram_tensor("wvv", [128, ET * H * D], f8, kind="ExternalInput").ap()
    projw = nc.dram_tensor("projw", [128, NP * E], bf16, kind="ExternalInput").ap()
    w1u = nc.dram_tensor("w1u", [128, FT * ET * 128], f8,
                         kind="ExternalInput").ap()
    w1v = nc.dram_tensor("w1v", [128, FT * ET * 128], f8,
                         kind="ExternalInput").ap()
    w2u = nc.dram_tensor("w2u", [128, FT * E], f8, kind="ExternalInput").ap()
    w2v = nc.dram_tensor("w2v", [128, FT * E], f8, kind="ExternalInput").ap()
    qb_d = nc.dram_tensor("qb", [128, ET], f32, kind="ExternalInput").ap()
    kb_d = nc.dram_tensor("kb", [128, ET], f32, kind="ExternalInput").ap()
    vb_d = nc.dram_tensor("vb", [128, H * D], bf16, kind="ExternalInput").ap()
    b1_d = nc.dram_tensor("b1", [128, FT], f32, kind="ExternalInput").ap()
    bf2_d = nc.dram_tensor("bf2b", [128, E], bf16, kind="ExternalInput").ap()
    maskE_d = nc.dram_tensor("maskE", [128, 256], bf16, kind="ExternalInput").ap()
    maskO_d = nc.dram_tensor("maskO", [128, 256], bf16, kind="ExternalInput").ap()
    out = nc.dram_tensor("out", [M, E], f32, kind="ExternalOutput").ap()

    with tile.TileContext(nc) as tc:
      for _rep in range(repeat):
        es = ExitStack()
        with es:
            # ---------- constants (whole kernel) ----------
            constp = es.enter_context(tc.tile_pool(name="const", bufs=1))
            ident = constp.tile([128, 128], bf16)
            cmasks.make_identity(nc, ident[:])
            maskE = constp.tile([128, 256], bf16)
            nc.sync.dma_start(maskE[:], maskE_d)
            maskO = constp.tile([128, 256], bf16)
            nc.sync.dma_start(maskO[:], maskO_d)
            qb = constp.tile([128, ET], f32)
            nc.sync.dma_start(qb[:], qb_d)
            kb = constp.tile([128, ET], f32)
            nc.sync.dma_start(kb[:], kb_d)
            vb = constp.tile([128, H * D], bf16)
            nc.sync.dma_start(vb[:], vb_d)
            b1 = constp.tile([128, FT], f32)
            nc.sync.dma_start(b1[:], b1_d)
            bf2 = constp.tile([128, E], bf16)
            nc.sync.dma_start(bf2[:], bf2_d)
            eps_t = constp.tile([128, 1], f32)
            nc.gpsimd.memset(eps_t[:], EPS)

            # helper: layernorm one 128-row chunk (fp32 src slice in SBUF) and
            # write the transposed bf16 result into dst_T[:, et, col:col+128].
            def ln_stats(src, statp):
                st = statp.tile([128, 2, 6], f32, tag="st")
                for g in range(2):
                    nc.vector.bn_stats(st[:, g, :], src[:, g * 512:(g + 1) * 512])
                ag = statp.tile([128, 2], f32, tag="ag")
                nc.vector.bn_aggr(ag[:], st[:])
                lv = statp.tile([128, 1], f32, tag="lv")
                nc.scalar.activation(lv[:], ag[:, 1:2], AF.Ln, bias=eps_t[:])
                rstd = statp.tile([128, 1], f32, tag="rstd")
                nc.scalar.activation(rstd[:], lv[:], AF.Exp, scale=-0.5)
                return ag, rstd

            def ln_apply(lc, src, ag, rstd, statp, use_act):
                # normalize-apply, split across two engines. use_act=True
                # (LN1 phase: ACT idle, DVE loaded): ACT Identity computes
                # (src*rstd - m*rstd) for half0, GPSIMD half1. use_act=False
                # (LN2, during attention: ACT saturated): DVE + GPSIMD.
                if use_act:
                    mr = statp.tile([128, 1], f32, tag="mr")
                    nc.vector.tensor_scalar(mr[:], ag[:, 0:1], rstd[:], -1.0,
                                            OP.mult, OP.mult)
                    nc.scalar.activation(lc[:, 0:512], src[:, 0:512],
                                         AF.Identity, bias=mr[:],
                                         scale=rstd[:])
                else:
                    nc.vector.tensor_scalar(lc[:, 0:512], src[:, 0:512],
                                            ag[:, 0:1], rstd[:],
                                            OP.subtract, OP.mult)
                nc.gpsimd.tensor_scalar(lc[:, 512:1024], src[:, 512:1024],
                                        ag[:, 0:1], rstd[:],
                                        OP.subtract, OP.mult)

            def ln_chunk(src, dst_T, col, statp, lnstage, tpsum, ci,
                         stats=None, dst8=None, col8=0, copies_act=True,
                         lc=None):
                if lc is None:
                    ag, rstd = (stats if stats is not None
                                else ln_stats(src, statp))
                    lc = lnstage.tile([128, E], bf16)
                    ln_apply(lc, src, ag, rstd, statp, copies_act)
                for et in range(ET):
                    tp = tpsum.tile([128, 128], bf16)
                    nc.tensor.transpose(tp[:],
                                        lc[:, et * 128:(et + 1) * 128],
                                        ident[:])
                    dst = dst_T[:, et, col:col + 128]
                    if copies_act and (et + ci) % 2 == 0:
                        nc.scalar.copy(dst, tp[:])
                    else:
                        nc.vector.tensor_copy(dst, tp[:])
                    # (copies stay off GPSIMD: it cannot read PSUM)
                if dst8 is not None:
                    # fp8 shadow for the Q/K DoubleRow rhs, off the busy
                    # engines: GPSIMD reads the finished bf16 strip. Two
                    # halves so downstream chains start at half-done.
                    nc.gpsimd.tensor_copy(dst8[:, 0:4, col8:col8 + 128],
                                          dst_T[:, 0:4, col:col + 128])
                    nc.gpsimd.tensor_copy(dst8[:, 4:8, col8:col8 + 128],
                                          dst_T[:, 4:8, col:col + 128])

            # ---------- scope B: qT/kT/v (strict stack nesting) ----------
            xmid = es.enter_context(tc.tile_pool(name="xmidp", bufs=1)).tile(
                [128, NS, E], bf16)
            # LN2 stats+apply run during the attention tail (DVE/GPSIMD);
            # only the transposes+copies stay in scope D. lc2 holds the
            # normalized (pre-transpose) chunks across the scope boundary.
            lc2 = es.enter_context(tc.tile_pool(name="ln2s", bufs=1)).tile(
                [128, NS, E], bf16)
            statp2o = es.enter_context(tc.tile_pool(name="statp2o", bufs=6))
            with ExitStack() as sB:
                qT = sB.enter_context(tc.tile_pool(name="qTp", bufs=1)).tile(
                    [128, NP, M], bf16)
                kT = sB.enter_context(tc.tile_pool(name="kTp", bufs=1)).tile(
                    [128, NP, T], bf16)
                # per-pair AV stationary operand [V_h0 | ones(64) | V_h1]:
                # head0 reads cols 0:128, head1 cols 64:192 — the shared ones
                # block makes the same matmul that accumulates attn@V also
                # accumulate the softmax denominator (replicated on the 64
                # out-rows opposite each head's data rows).
                # vS carries 32*v (fp8 wv split is stored x32); the ones
                # block becomes 32.0 so the denominator picks up the same
                # scale and do_norm's divide cancels it exactly.
                vS = sB.enter_context(tc.tile_pool(name="vp", bufs=1)).tile(
                    [128, TK, NP, 192], bf16)

                # ---------- scope A: LN1 + QKV projections ----------
                with ExitStack() as sA:
                    wpool = sA.enter_context(tc.tile_pool(name="wpool", bufs=1))
                    stage = sA.enter_context(tc.tile_pool(name="xstage", bufs=4))
                    lnstage = sA.enter_context(tc.tile_pool(name="lnstage", bufs=2))
                    statp = sA.enter_context(tc.tile_pool(name="statp", bufs=6))
                    tpsum = sA.enter_context(
                        tc.tile_pool(name="tpsum", bufs=4, space="PSUM"))
                    qps = sA.enter_context(
                        tc.tile_pool(name="qps", bufs=2, space="PSUM"))

                    # lnf chunks rotate through a 3-deep stage: V consumes
                    # each chunk as soon as it lands; Q/K read the persistent
                    # fp8 shadow lnf8.
                    lnfp = sA.enter_context(tc.tile_pool(name="lnfp", bufs=4))
                    lnf8 = sA.enter_context(tc.tile_pool(name="lnf8p", bufs=1)).tile(
                        [128, ET, T], f8)
                    # LN1 over x_full (host-permuted: q rows are chunks 0..7,
                    # the pair-core's rows are chunks 8..15) -> lnf. Q^T, each
                    # K^T token-quarter and each V chunk are emitted as soon
                    # as the lnf region they read is complete, so the PE
                    # works through projections while the LN chains run.
                    wq_sb = wpool.tile([128, ET, H * D], f8, tag="wq")
                    wk_sb = wpool.tile([128, ET, H * D], f8, tag="wk")
                    wvu_sb = wpool.tile([128, ET, H * D], f8, tag="wvu")
                    wvv_sb = wpool.tile([128, ET, H * D], f8, tag="wvv")
                    nc.sync.dma_start(wvu_sb[:], wvu.rearrange(
                        "p (et n) -> p et n", et=ET))
                    nc.sync.dma_start(wvv_sb[:], wvv.rearrange(
                        "p (et n) -> p et n", et=ET))
                    lnbp = sA.enter_context(tc.tile_pool(name="lnbp", bufs=3))

                    def v_chunk(t, lnfc):
                        # split-fp8: v = a@u + b@u + a@v with a=fp8(lnf),
                        # b=fp8(lnf-a), u/v the fp8 split of 32*wv. 12 DR
                        # matmuls replace 16 bf16 ones (0.75x PE).
                        a8 = lnf8[:, :, t * 128:(t + 1) * 128]
                        b8 = lnbp.tile([128, ET, 128], f8, tag="lnb")
                        nc.vector.tensor_tensor(b8[:, 0:4], lnfc[:, 0:4],
                                                a8[:, 0:4], OP.subtract)
                        nc.gpsimd.tensor_tensor(b8[:, 4:8], lnfc[:, 4:8],
                                                a8[:, 4:8], OP.subtract)
                        ps = qps.tile([128, 1024], f32)
                        for hc in range(2):
                            cols = slice(hc * 512, (hc + 1) * 512)
                            chains = [(a8, wvu_sb), (a8, wvv_sb),
                                      (b8, wvu_sb)]
                            for ci, (act, wt) in enumerate(chains):
                                for ep in range(ET // 2):
                                    nc.tensor.matmul(
                                        ps[:, cols],
                                        lhsT=act[:, 2 * ep:2 * ep + 2, :],
                                        rhs=wt[:, 2 * ep:2 * ep + 2, cols],
                                        start=(ci == 0 and ep == 0),
                                        stop=(ci == 2 and ep == ET // 2 - 1),
                                        perf_mode=DR)
                        psv = ps[:].rearrange("p (np h d) -> p np h d", np=NP,
                                              h=2)
                        vbv = vb[:].rearrange("p (np h d) -> p np h d", np=NP,
                                              h=2)
                        nc.vector.tensor_add(
                            vS[:, t, :, 0:64], psv[:, :, 0, :], vbv[:, :, 0, :])
                        nc.vector.tensor_add(
                            vS[:, t, :, 128:192], psv[:, :, 1, :],
                            vbv[:, :, 1, :])

                    def k_quarter(tq, ms):
                        for m in ms:
                            ps = qps.tile([128, 512], f32)
                            for ep in range(ET // 2):
                                nc.tensor.matmul(
                                    ps[:],
                                    lhsT=wk_sb[:, 2 * ep:2 * ep + 2,
                                               m * 128:(m + 1) * 128],
                                    rhs=lnf8[:, 2 * ep:2 * ep + 2,
                                             tq * 512:(tq + 1) * 512],
                                    start=(ep == 0), stop=(ep == ET // 2 - 1),
                                    perf_mode=DR)
                            # descale+bias on ACT (idle in this phase)
                            nc.scalar.activation(
                                kT[:, m, tq * 512:(tq + 1) * 512], ps[:],
                                AF.Identity, bias=kb[:, m:m + 1],
                                scale=1.0 / 32)

                    def q_proj(ms):
                        for m in ms:
                            ps = qps.tile([128, 1024], f32)
                            for qc in range(2):
                                for ep in range(ET // 2):
                                    nc.tensor.matmul(
                                        ps[:, qc * 512:(qc + 1) * 512],
                                        lhsT=wq_sb[:, 2 * ep:2 * ep + 2,
                                                   m * 128:(m + 1) * 128],
                                        rhs=lnf8[:, 2 * ep:2 * ep + 2,
                                                 qc * 512:(qc + 1) * 512],
                                        start=(ep == 0),
                                        stop=(ep == ET // 2 - 1),
                                        perf_mode=DR)
                            nc.scalar.activation(
                                qT[:, m, :], ps[:], AF.Identity,
                                bias=qb[:, m:m + 1],
                                scale=float(E) ** -0.5 / 32)

                    for c in range(TK):
                        xc = stage.tile([128, E], bf16)
                        nc.scalar.dma_start(
                            xc[:], x_full[c * 128:(c + 1) * 128, :])
                        lnfc = lnfp.tile([128, ET, 128], bf16, tag="lnfc")
                        ln_chunk(xc[:], lnfc, 0, statp, lnstage,
                                 tpsum, c, dst8=lnf8, col8=c * 128)
                        if c == 0:
                            nc.sync.dma_start(wk_sb[:], wk.rearrange(
                                "p (et n) -> p et n", et=ET))
                        if c == 1:
                            nc.sync.dma_start(wq_sb[:], wq.rearrange(
                                "p (et n) -> p et n", et=ET))
                        v_chunk(c, lnfc)
                        if c % 4 == 3:
                            k_quarter(c // 4, range(0, 4))
                        if c % 4 == 0 and c > 0:
                            k_quarter(c // 4 - 1, range(4, ET))
                        if c == TK - 1:
                            k_quarter(3, range(4, ET))
                        if c == NS - 1:
                            q_proj(range(0, 4))
                        if c == NS + 1:
                            q_proj(range(4, ET))

                # ---------- attention + output projection ----------
                # half-outer: all pairs finish q-cols [0,512) first; the
                # output projection for q-chunks 0..3 is then dripped one
                # chunk at a time between second-half pairs so the PE works
                # through proj while ACT drains the exp/normalize backlog.
                with ExitStack() as sC:
                    oT = sC.enter_context(tc.tile_pool(name="oTp", bufs=1)).tile(
                        [128, NP, M], bf16)
                    ptp = sC.enter_context(tc.tile_pool(name="ptp", bufs=5))
                    normp = sC.enter_context(tc.tile_pool(name="normp", bufs=2))
                    xqpp = sC.enter_context(tc.tile_pool(name="xqpp", bufs=1))
                    pwp = sC.enter_context(tc.tile_pool(name="pwp", bufs=1))
                    apsum = sC.enter_context(
                        tc.tile_pool(name="apsum", bufs=2, space="PSUM"))
                    spsum = sC.enter_context(
                        tc.tile_pool(name="spsum", bufs=2, space="PSUM"))

                    maskEv = maskE[:].rearrange("p (h q) -> p h q", h=2)
                    maskOv = maskO[:].rearrange("p (h q) -> p h q", h=2)

                    pw_sb = pwp.tile([128, NP, E], bf16)
                    nc.sync.dma_start(pw_sb[:], projw.rearrange(
                        "p (m e) -> p m e", m=NP))
                    nc.gpsimd.memset(vS[:, 0:8, :, 64:128], 32.0)
                    nc.gpsimd.memset(vS[:, 8:16, :, 64:128], 32.0)
                    xq_t = {}

                    def do_norm(av, p, half):
                        # den sits on the 64 rows opposite each head's data;
                        # 1/den via the fast DVE Newton-Raphson reciprocal
                        # (dens are sums of exps, well inside its safe range)
                        # keeps the whole normalize off the exp-saturated ACT.
                        # one full-tile fast reciprocal (custom DVE ops
                        # require partition base 0; the data-row lanes are
                        # junk and never read)
                        rcp = normp.tile([128, 2, 512], f32, tag="rcp")
                        nc.vector.reciprocal_approx_fast(rcp[:], av[:])
                        colr = slice(512 * half, 512 * (half + 1))
                        for h in range(2):
                            dn = slice(64 * (1 - h), 64 * (1 - h) + 64)
                            nc.vector.tensor_mul(
                                oT[64 * h:64 * h + 64, p, colr],
                                av[64 * h:64 * h + 64, h, :],
                                rcp[dn, h, :])

                    # permuted key order: chunks 0..7 are this core's parity
                    # (incl. the causal diagonal), 8..15 the pair-core's
                    # (strictly past or future, selected by the all-ones/
                    # all-zero parity mask).
                    CHUNKS = [
                        [(kt, 128 * kt, "tri") for kt in range(4)] +
                        [(kt, 128 * (kt - 8), "par") for kt in range(8, 12)],
                        [(kt, 0, None) for kt in range(4)] +
                        [(kt, 128 * (kt - 4), "tri") for kt in range(4, 8)] +
                        [(kt, 0, None) for kt in range(8, 12)] +
                        [(kt, 128 * (kt - 12), "par") for kt in range(12, 16)],
                    ]

                    def do_av(item):
                        av, p, half, kt, qlo, sp, pt = item
                        for h in range(2):
                            nc.tensor.matmul(
                                av[:, h, qlo:512],
                                lhsT=vS[:, kt, p, 64 * h:64 * h + 128],
                                rhs=pt[:, h, qlo:512],
                                start=(kt == 0), stop=sp,
                                skip_group_check=True)
                        if sp:
                            do_norm(av, p, half)

                    def proj_qm(qm):
                        if qm // 4 not in xq_t:
                            xqh = xqpp.tile([128, 4, E], bf16, tag="xq")
                            xq_t[qm // 4] = xqh
                        xqh = xq_t[qm // 4]
                        nc.sync.dma_start(
                            xqh[:, qm % 4, :], xqp[qm * 128:(qm + 1) * 128, :])
                        ps = apsum.tile([128, 1024], f32, tag="av")
                        for ec in range(2):
                            for pk in range(NP):
                                nc.tensor.matmul(
                                    ps[:, ec * 512:(ec + 1) * 512],
                                    lhsT=oT[:, pk, qm * 128:(qm + 1) * 128],
                                    rhs=pw_sb[:, pk, ec * 512:(ec + 1) * 512],
                                    start=(pk == 0), stop=(pk == NP - 1))
                        nc.vector.tensor_add(
                            xmid[:, qm, :], ps[:], xqh[:, qm % 4, :])

                    def ln2_sa(qm):
                        ag, rstd = ln_stats(xmid[:, qm, :], statp2o)
                        ln_apply(lc2[:, qm, :], xmid[:, qm, :], ag, rstd,
                                 statp2o, False)
                        # after LN2 consumed xmid, fold the final bf2 bias in
                        nc.gpsimd.tensor_add(xmid[:, qm, :], xmid[:, qm, :],
                                             bf2[:])

                    def attn_half(half, interleave=()):
                        chunks = CHUNKS[half]
                        last_kt = chunks[-1][0]
                        pend = []
                        for p in range(NP):
                            av = apsum.tile([128, 2, 512], f32, tag="av")
                            for kt, qlo, mk in chunks:
                                ps = spsum.tile([128, 2, 512], f32)
                                for h in range(2):
                                    nc.tensor.matmul(
                                        ps[:, h, qlo:512],
                                        lhsT=kT[64 * h:64 * h + 64, p,
                                                kt * 128:(kt + 1) * 128],
                                        rhs=qT[64 * h:64 * h + 64, p,
                                               512 * half + qlo:
                                               512 * (half + 1)],
                                        start=True, stop=True)
                                pt = ptp.tile([128, 2, 512], bf16)
                                nc.scalar.activation(
                                    pt[:, :, qlo:512], ps[:, :, qlo:512],
                                    AF.Exp)
                                if mk is not None:
                                    mkv = maskEv if mk == "tri" else maskOv
                                    nc.vector.tensor_mul(
                                        pt[:, :, qlo:qlo + 128],
                                        pt[:, :, qlo:qlo + 128], mkv)
                                pend.append((av, p, half, kt, qlo,
                                             kt == last_kt, pt))
                                if len(pend) > 4:
                                    do_av(pend.pop(0))
                            if p >= 1 and p - 1 < len(interleave):
                                while len(pend) > 4:
                                    do_av(pend.pop(0))
                                interleave[p - 1]()
                        while pend:
                            do_av(pend.pop(0))

                    attn_half(0)
                    tasks = []
                    for qm in range(4):
                        tasks.append(lambda qm=qm: proj_qm(qm))
                        tasks.append(lambda qm=qm: ln2_sa(qm))
                    attn_half(1, interleave=tasks[:7])
                    tasks[7]()
                    for qm in range(4, NS):
                        proj_qm(qm)
                        ln2_sa(qm)

            # ---------- scope D: LN2 + FFN ----------
            # LN2 q-chunks 0..3 interleave with the tail projections; FFN1's
            # first-half fm groups interleave with LN2 chunks 4..7 so the
            # PE never drains while DVE/ACT run the layernorm chains.
            with ExitStack() as sD:
                ln2p = sD.enter_context(tc.tile_pool(name="ln2p", bufs=1))
                ln2a = ln2p.tile([128, ET, M], f8, tag="a")
                ln2b = ln2p.tile([128, ET, M], f8, tag="b")
                w2p = sD.enter_context(tc.tile_pool(name="w2p", bufs=1))
                w2u_sb = w2p.tile([128, FT, E], f8, tag="u")
                w2v_sb = w2p.tile([128, FT, E], f8, tag="v")

                tpsum2 = sD.enter_context(
                    tc.tile_pool(name="tpsum2", bufs=2, space="PSUM"))
                rtp = sD.enter_context(tc.tile_pool(name="rtp", bufs=1))
                rbfp = sD.enter_context(tc.tile_pool(name="rbfp", bufs=3))
                w1p = sD.enter_context(tc.tile_pool(name="w1p", bufs=6))
                zps = sD.enter_context(
                    tc.tile_pool(name="zps", bufs=2, space="PSUM"))
                ops = sD.enter_context(
                    tc.tile_pool(name="ops", bufs=2, space="PSUM"))
                outp = sD.enter_context(tc.tile_pool(name="outp", bufs=3))

                def ln2_qm(qm):
                    # stats+apply already ran in the attention tail. Here:
                    # transpose (PE), then the fp8 split stores — a on ACT,
                    # b = tp - a on DVE.
                    col = qm * 128
                    lc = lc2[:, qm, :]
                    for et in range(ET):
                        tp = tpsum2.tile([128, 128], bf16)
                        nc.tensor.transpose(tp[:],
                                            lc[:, et * 128:(et + 1) * 128],
                                            ident[:])
                        nc.scalar.copy(ln2a[:, et, col:col + 128], tp[:])
                        nc.vector.tensor_tensor(
                            ln2b[:, et, col:col + 128], tp[:],
                            ln2a[:, et, col:col + 128], OP.subtract)

                rT_tiles = {}

                def ffn1_fm(half, fm):
                    if half not in rT_tiles:
                        ra = rtp.tile([128, FT, 512], f8, tag="rTa")
                        rb = rtp.tile([128, FT, 512], f8, tag="rTb")
                        rT_tiles[half] = (ra, rb)
                    ra, rb = rT_tiles[half]
                    w1f = w1p.tile([128, 2, ET, 128], f8)
                    nc.sync.dma_start(
                        w1f[:, 0], w1u.rearrange("p (fm et f) -> p fm et f",
                                                 fm=FT, et=ET)[:, fm])
                    nc.sync.dma_start(
                        w1f[:, 1], w1v.rearrange("p (fm et f) -> p fm et f",
                                                 fm=FT, et=ET)[:, fm])
                    zp = zps.tile([128, 512], f32)
                    cols = slice(half * 512, (half + 1) * 512)
                    chains = [(ln2a, 0), (ln2a, 1), (ln2b, 0)]
                    for ci, (act, wi) in enumerate(chains):
                        for ep in range(ET // 2):
                            nc.tensor.matmul(
                                zp[:],
                                lhsT=w1f[:, wi, 2 * ep:2 * ep + 2, :],
                                rhs=act[:, 2 * ep:2 * ep + 2, cols],
                                start=(ci == 0 and ep == 0),
                                stop=(ci == 2 and ep == ET // 2 - 1),
                                perf_mode=DR)
                    # relu (scale descales the x32 of w1), then fp8 split:
                    # a copy on ACT, b = r - a on DVE.
                    # rbf = relu(z + 32*b1) = 32*relu(z/32+b1); the x32
                    # rides the residual stream (host-descaled x2048).
                    rbf = rbfp.tile([128, 512], bf16)
                    nc.scalar.activation(rbf[:], zp[:], AF.Relu,
                                         bias=b1[:, fm:fm + 1])
                    # fp8 split: a on the idle GPSIMD (SBUF->SBUF), b on DVE
                    nc.gpsimd.tensor_copy(ra[:, fm, :], rbf[:])
                    nc.vector.tensor_tensor(rb[:, fm, :], rbf[:],
                                            ra[:, fm, :], OP.subtract)

                def ffn2_qq(half, qq):
                    ra, rb = rT_tiles[half]
                    qm = half * 4 + qq
                    ot = outp.tile([128, E], f32)
                    op = ops.tile([128, 1024], f32)
                    for ec in range(2):
                        cols = slice(ec * 512, (ec + 1) * 512)
                        chains = [(ra, w2u_sb), (ra, w2v_sb), (rb, w2u_sb)]
                        for ci, (act, wt) in enumerate(chains):
                            for fp_ in range(FT // 2):
                                nc.tensor.matmul(
                                    op[:, cols],
                                    lhsT=act[:, 2 * fp_:2 * fp_ + 2,
                                             qq * 128:(qq + 1) * 128],
                                    rhs=wt[:, 2 * fp_:2 * fp_ + 2, cols],
                                    start=(ci == 0 and fp_ == 0),
                                    stop=(ci == 2 and fp_ == FT // 2 - 1),
                                    perf_mode=DR)
                        nc.vector.tensor_add(
                            ot[:, cols], op[:, cols], xmid[:, qm, cols])
                        nc.scalar.dma_start(
                            out[qm * 128:(qm + 1) * 128, cols], ot[:, cols])

                for qm in range(4):
                    ln2_qm(qm)
                # LN2 chunks 4..7 drip between the first 4 fm-groups of
                # FFN1-half0 (which only needs ln2T token cols 0:512).
                for g in range(4):
                    ln2_qm(4 + g)
                    nc.sync.dma_start(
                        w2u_sb[:, g * 8:(g + 1) * 8, :],
                        w2u.rearrange("p (ft e) -> p ft e",
                                      ft=FT)[:, g * 8:(g + 1) * 8, :])
                    nc.sync.dma_start(
                        w2v_sb[:, g * 8:(g + 1) * 8, :],
                        w2v.rearrange("p (ft e) -> p ft e",
                                      ft=FT)[:, g * 8:(g + 1) * 8, :])
                    for fm in range(g * 8, g * 8 + 8):
                        ffn1_fm(0, fm)
                for qq in range(4):
                    ffn2_qq(0, qq)
                rT_tiles.pop(0)
                for fm in range(FT):
                    ffn1_fm(1, fm)
                for qq in range(4):
                    ffn2_qq(1, qq)

    nc.compile()
    _CACHE[key] = nc
    return nc


def _prep_inputs(x, wq, wk, wv, proj_w, proj_b, g1, beta1, g2, beta2, w1, bf1,
                 w2, bf2):
    """Host-side sharding + weight folding. Returns list of 8 in_maps."""
    f32 = np.float32
    x = np.asarray(x, f32)
    scale = float(E) ** -0.5

    Wq = np.asarray(wq, f32).transpose(1, 0, 2).reshape(E, H * D)
    Wk = np.asarray(wk, f32).transpose(1, 0, 2).reshape(E, H * D)
    Wv = np.asarray(wv, f32).transpose(1, 0, 2).reshape(E, H * D)
    g1 = np.asarray(g1, f32)
    beta1 = np.asarray(beta1, f32)
    g2 = np.asarray(g2, f32)
    beta2 = np.asarray(beta2, f32)
    w1 = np.asarray(w1, f32)
    w2 = np.asarray(w2, f32)
    bf1 = np.asarray(bf1, f32)
    bf2 = np.asarray(bf2, f32)
    proj_w = np.asarray(proj_w, f32)
    proj_b = np.asarray(proj_b, f32)

    def sb_layout(w, ntile):
        # [ntile*128, N] -> [128, ntile*N] with per-partition contiguous tiles
        n = w.shape[1]
        return np.ascontiguousarray(
            w.reshape(ntile, 128, n).transpose(1, 0, 2).reshape(128, ntile * n))

    # fp8 weight scaling: x32 (x64 for w2) puts sigma at ~1 inside e4m3's
    # normal range. V/FFN weights are double-split (u = fp8(w*s),
    # v = fp8(w*s - u)) for the 3-chain split matmuls; Q/K use u only.
    def split8(w, s):
        u = (w * s).astype(F8)
        v = ((w * s) - u.astype(f32)).astype(F8)
        return u, v

    wq_b = sb_layout((Wq * g1[:, None] * 32).astype(F8), ET)
    wk_b = sb_layout((Wk * g1[:, None] * 32).astype(F8), ET)
    wvu_n, wvv_n = split8(Wv * g1[:, None], 32)
    wvu_b, wvv_b = sb_layout(wvu_n, ET), sb_layout(wvv_n, ET)
    qbias = (beta1 @ Wq) * scale
    kbias = beta1 @ Wk
    vbias = (beta1 @ Wv) * 32
    w1u_n, w1v_n = split8(w1 * g2[:, None], 32)

    def w1_layout(w):
        return np.ascontiguousarray(
            w.reshape(ET, 128, FT, 128).transpose(1, 2, 0, 3)
            .reshape(128, FT * ET * 128))

    w1u_b, w1v_b = w1_layout(w1u_n), w1_layout(w1v_n)
    b1v = bf1 + beta2 @ w1
    w2u_n, w2v_n = split8(w2, 64)
    w2u_b, w2v_b = sb_layout(w2u_n, FT), sb_layout(w2v_n, FT)
    # the residual stream runs x64 on-chip (w2 split carries it; proj_w and
    # xqp are pre-scaled to match); divided back out on the host.
    projw_b = sb_layout((proj_w * 2048).astype(BF16), NP)

    qb = np.ascontiguousarray(qbias.reshape(ET, 128).T, f32)
    kb = np.ascontiguousarray(kbias.reshape(ET, 128).T, f32)
    vb = np.ascontiguousarray(np.broadcast_to(vbias, (128, H * D))).astype(BF16)
    b1m = np.ascontiguousarray(b1v.reshape(FT, 128).T * 32, f32)
    bf2m = np.ascontiguousarray(
        np.broadcast_to(bf2 * 2048, (128, E))).astype(BF16)

    tri = np.triu(np.ones((128, 128), f32))  # [k_row, q_col]: 1 iff k <= q
    zerosm = np.zeros((128, 128), f32)
    # maskE = causal diagonal (all cores); maskO = parity: the pair-core's
    # diagonal-adjacent chunk is strictly past (odd cores) or future (even).
    mO = {0: zerosm, 1: tri * 0 + 1}
    in_maps = []
    for c in range(NCORES):
        b, hpar = c // 2, c % 2
        xc = x[b].reshape(TK, 128, E)
        xq = np.ascontiguousarray(xc[hpar::2].reshape(M, E), f32)
        xperm = np.ascontiguousarray(
            np.concatenate([xc[hpar::2], xc[1 - hpar::2]], axis=0)
            .reshape(T, E)).astype(BF16)
        in_maps.append({
            "x_full": xperm,
            "xqp": ((xq + proj_b[None, :].astype(f32)) * 2048).astype(BF16),
            "wq": wq_b, "wk": wk_b, "wvu": wvu_b, "wvv": wvv_b,
            "projw": projw_b, "w1u": w1u_b, "w1v": w1v_b,
            "w2u": w2u_b, "w2v": w2v_b,
            "qb": qb, "kb": kb, "vb": vb, "b1": b1m, "bf2b": bf2m,
            "maskE": np.ascontiguousarray(
                np.tile(tri, (1, 2))).astype(BF16),
            "maskO": np.ascontiguousarray(
                np.tile(mO[hpar], (1, 2))).astype(BF16),
        })
    return in_maps


def _run(inputs, trace=False):
    from concourse.bass_utils import run_bass_kernel_spmd
    nc = _build()
    in_maps = _prep_inputs(**inputs)
    res = run_bass_kernel_spmd(nc, in_maps, core_ids=list(range(NCORES)),
                               trace=trace)
    full = np.empty((B, T, E), np.float32)
    for c in range(NCORES):
        b, hpar = c // 2, c % 2
        full[b].reshape(TK, 128, E)[hpar::2] = (
            res.results[c]["out"].reshape(NS, 128, E) * np.float32(1 / 2048))
    return full, res


def kernel(**inputs) -> np.ndarray:
    out, _ = _run(inputs, trace=False)
    return out



# revision 127
# speedup vs baseline: 1.0150x; 1.0083x over previous
# Trainium2 Bass kernel for nn_DecoderBlock (B=4, T=2048, E=1024, H=16, D=64, FF=4096).
#
# Sharding: 8-way data parallel, zero collectives. Core c = 2*b + h handles batch b
# and the interleaved half of the sequence: 128-row q-blocks {2s+h : s=0..7}
# (1024 q rows per core). K/V are computed per-core for the full T=2048 rows of its
# batch (duplicated across the two cores of a batch pair) so attention needs no
# cross-core communication. The interleaved block assignment makes the causal
# work pattern identical on every core (uniform SPMD program): q-slot s statically
# attends keys [0, 256*(s+1)), with a per-core {0,1} multiplicative mask (input
# data) handling the parity-dependent diagonal.
#
# On-chip layout: activations flow feature-major (S^T = [k, q]) through attention so
# softmax needs no transposes of the probability matrix. Softmax uses no max
# subtraction (scores are ~N(0, 0.25^2) by construction); 1/denominator is
# exp(-ln(d)) so the whole kernel uses one ACT table set (exp/ln/relu).
# fp8e4m3 DoubleRow matmuls (two 128-deep k-planes per instruction, 0.5 PE
# cycles/row = 4x bf16):
#  - Q/K projections: single-chain fp8 (softmax absorbs score quantization,
#    measured numerically free). rhs is an fp8 shadow of the LN1 output
#    (lnf8), copied from bf16 by the otherwise-idle GPSIMD engine.
#  - V / FFN1 / FFN2: double-split fp8, out = a@u + a@v + b@u with
#    a=fp8(act), b=fp8(act-a), u=fp8(w*s), v=fp8(w*s-u): 3 DR chains = 0.75x
#    bf16 PE time at ~bf16 accuracy (pure fp8 on any of these costs ~1.5e-2
#    max-rel against the 2e-2 gate; the split costs ~1e-3). The b-chain is
#    ordered last so its residual lands off the critical path.
#  - proj + attention scores/AV stay bf16.
# All descales are free by construction: weight scales are powers of two
# (x32, x64 for w2); wv's x32 makes vS carry 32*v and the ones-block is
# memset to 32.0 so the softmax denominator cancels it in do_norm; proj_w
# and xqp are host-scaled x64 so the whole post-attention residual stream
# runs x64 on-chip (LN2 is scale-invariant) and the host divides the final
# output by 64 after the gather; the FFN1 descale rides the relu's scale
# immediate. Softmax denominators invert on DVE via reciprocal_approx_fast
# (full-tile: custom DVE ops require partition base 0 — base-64 slices
# mis-execute on hardware). LN2 stats+apply run inside the attention tail on
# DVE/GPSIMD; only transposes + fp8 split stores remain in the FFN bridge.
# Matmuls accumulate in fp32 PSUM; LN statistics and the final output are
# fp32. LN gains (g1,g2) and the attention 1/sqrt(E) scale are folded into
# the weights on the host; beta terms become per-feature biases.

import numpy as np
import ml_dtypes
from contextlib import ExitStack

BF16 = ml_dtypes.bfloat16
F8 = ml_dtypes.float8_e4m3

B, T, E, H, D, FF = 4, 2048, 1024, 16, 64, 4096
M = 1024          # q rows per core
NCORES = 8
NS = 8            # q slots (128 rows) per core
ET = E // 128     # 8 e-tiles
TK = T // 128     # 16 k-tiles
FT = FF // 128    # 32 ff-tiles
NP = H // 2       # 8 head pairs
EPS = 1e-5

_CACHE = {}


def _build(repeat=1):
    """Build (and cache) the Bass module for one core's uniform program.

    repeat>1 emits the whole body N times (identical I/O) — used only for
    slope-based wall-clock timing of one body on hardware.
    """
    key = ("nc", repeat)
    if key in _CACHE:
        return _CACHE[key]

    import concourse.bacc as bacc
    import concourse.tile as tile
    import concourse.mybir as mybir
    from concourse import masks as cmasks

    dt = mybir.dt
    f32, bf16, f8 = dt.float32, dt.bfloat16, dt.float8e4
    AF = mybir.ActivationFunctionType
    OP = mybir.AluOpType
    DR = mybir.MatmulPerfMode.DoubleRow

    nc = bacc.Bacc("TRN2", target_bir_lowering=False, debug=False,
                   num_devices=NCORES)

    # Every activation we use (Exp, Ln, Relu, Copy, Identity) lives in the
    # 'natural_log_exp_and_others' table set. The default per-function set
    # choice alternates home sets (exp_and_others vs natural_log), inserting
    # ~80 ACT table loads (~100us). Restrict the chooser to the one set that
    # covers everything -> a single load.
    import types
    import bass_rust as _br

    def _insert_act_loads_one_set(self):
        has_activation = any(
            isinstance(i, mybir.InstActivation)
            for b in self.main_func.blocks for i in b.instructions)
        if not has_activation:
            return
        tabs = bacc.get_activation_tables(self.m.arch)
        ours = {mybir.ActivationFunctionType.Exp, mybir.ActivationFunctionType.Ln,
                mybir.ActivationFunctionType.Relu, mybir.ActivationFunctionType.Copy,
                mybir.ActivationFunctionType.Identity}
        filt = []
        for name, fns in tabs.items():
            if name == "natural_log_exp_and_others":
                assert ours <= fns
                filt.append((name, fns))
            else:
                filt.append((name, fns - ours))
        _br.insert_act_table_loads(self, filt)

    nc.insert_act_table_loads = types.MethodType(_insert_act_loads_one_set, nc)

    # ----- DRAM I/O -----
    x_full = nc.dram_tensor("x_full", [T, E], bf16, kind="ExternalInput").ap()
    xqp = nc.dram_tensor("xqp", [M, E], bf16, kind="ExternalInput").ap()
    # weights arrive pre-arranged on the host into SBUF layout
    # [128 partitions, <tile dims>] so each DMA is one long contiguous run
    # per partition (minimal descriptor count).
    wq = nc.dram_tensor("wq", [128, ET * H * D], f8, kind="ExternalInput").ap()
    wk = nc.dram_tensor("wk", [128, ET * H * D], f8, kind="ExternalInput").ap()
    wvu = nc.dram_tensor("wvu", [128, ET * H * D], f8, kind="ExternalInput").ap()
    wvv = nc.dram_tensor("wvv", [128, ET * H * D], f8, kind="ExternalInput").ap()
    projw = nc.dram_tensor("projw", [128, NP * E], bf16, kind="ExternalInput").ap()
    w1u = nc.dram_tensor("w1u", [128, FT * ET * 128], f8,
                         kind="ExternalInput").ap()
    w1v = nc.dram_tensor("w1v", [128, FT * ET * 128], f8,
                         kind="ExternalInput").ap()
    w2u = nc.dram_tensor("w2u", [128, FT * E], f8, kind="ExternalInput").ap()
    w2v = nc.dram_tensor("w2v", [128, FT * E], f8, kind="ExternalInput").ap()
    qb_d = nc.dram_tensor("qb", [128, ET], f32, kind="ExternalInput").ap()
    kb_d = nc.dram_tensor("kb", [128, ET], f32, kind="ExternalInput").ap()
    vb_d = nc.dram_tensor("vb", [128, H * D], bf16, kind="ExternalInput").ap()
    b1_d = nc.dram_tensor("b1", [128, FT], f32, kind="ExternalInput").ap()
    bf2_d = nc.dram_tensor("bf2b", [128, E], bf16, kind="ExternalInput").ap()
    maskE_d = nc.dram_tensor("maskE", [128, 256], bf16, kind="ExternalInput").ap()
    maskO_d = nc.dram_tensor("maskO", [128, 256], bf16, kind="ExternalInput").ap()
    out = nc.dram_tensor("out", [M, E], f32, kind="ExternalOutput").ap()

    with tile.TileContext(nc) as tc:
      for _rep in range(repeat):
        es = ExitStack()
        with es:
            # ---------- constants (whole kernel) ----------
            constp = es.enter_context(tc.tile_pool(name="const", bufs=1))
            ident = constp.tile([128, 128], bf16)
            cmasks.make_identity(nc, ident[:])
            maskE = constp.tile([128, 256], bf16)
            nc.sync.dma_start(maskE[:], maskE_d)
            maskO = constp.tile([128, 256], bf16)
            nc.sync.dma_start(maskO[:], maskO_d)
            qb = constp.tile([128, ET], f32)
            nc.sync.dma_start(qb[:], qb_d)
            kb = constp.tile([128, ET], f32)
            nc.sync.dma_start(kb[:], kb_d)
            vb = constp.tile([128, H * D], bf16)
            nc.sync.dma_start(vb[:], vb_d)
            b1 = constp.tile([128, FT], f32)
            nc.sync.dma_start(b1[:], b1_d)
            bf2 = constp.tile([128, E], bf16)
            nc.sync.dma_start(bf2[:], bf2_d)
            eps_t = constp.tile([128, 1], f32)
            nc.gpsimd.memset(eps_t[:], EPS)

            # helper: layernorm one 128-row chunk (fp32 src slice in SBUF) and
            # write the transposed bf16 result into dst_T[:, et, col:col+128].
            def ln_stats(src, statp):
                st = statp.tile([128, 2, 6], f32, tag="st")
                for g in range(2):
                    nc.vector.bn_stats(st[:, g, :], src[:, g * 512:(g + 1) * 512])
                ag = statp.tile([128, 2], f32, tag="ag")
                nc.vector.bn_aggr(ag[:], st[:])
                lv = statp.tile([128, 1], f32, tag="lv")
                nc.scalar.activation(lv[:], ag[:, 1:2], AF.Ln, bias=eps_t[:])
                rstd = statp.tile([128, 1], f32, tag="rstd")
                nc.scalar.activation(rstd[:], lv[:], AF.Exp, scale=-0.5)
                return ag, rstd

            def ln_apply(lc, src, ag, rstd, statp, use_act):
                # normalize-apply, split across two engines. use_act=True
                # (LN1 phase: ACT idle, DVE loaded): ACT Identity computes
                # (src*rstd - m*rstd) for half0, GPSIMD half1. use_act=False
                # (LN2, during attention: ACT saturated): DVE + GPSIMD.
                if use_act:
                    mr = statp.tile([128, 1], f32, tag="mr")
                    nc.vector.tensor_scalar(mr[:], ag[:, 0:1], rstd[:], -1.0,
                                            OP.mult, OP.mult)
                    nc.scalar.activation(lc[:, 0:512], src[:, 0:512],
                                         AF.Identity, bias=mr[:],
                                         scale=rstd[:])
                else:
                    nc.vector.tensor_scalar(lc[:, 0:512], src[:, 0:512],
                                            ag[:, 0:1], rstd[:],
                                            OP.subtract, OP.mult)
                nc.gpsimd.tensor_scalar(lc[:, 512:1024], src[:, 512:1024],
                                        ag[:, 0:1], rstd[:],
                                        OP.subtract, OP.mult)

            def ln_chunk(src, dst_T, col, statp, lnstage, tpsum, ci,
                         stats=None, dst8=None, col8=0, copies_act=True,
                         lc=None):
                if lc is None:
                    ag, rstd = (stats if stats is not None
                                else ln_stats(src, statp))
                    lc = lnstage.tile([128, E], bf16)
                    ln_apply(lc, src, ag, rstd, statp, copies_act)
                for et in range(ET):
                    tp = tpsum.tile([128, 128], bf16)
                    nc.tensor.transpose(tp[:],
                                        lc[:, et * 128:(et + 1) * 128],
                                        ident[:])
                    dst = dst_T[:, et, col:col + 128]
                    if copies_act and (et + ci) % 2 == 0:
                        nc.scalar.copy(dst, tp[:])
                    else:
                        nc.vector.tensor_copy(dst, tp[:])
                    # (copies stay off GPSIMD: it cannot read PSUM)
                if dst8 is not None:
                    # fp8 shadow for the Q/K DoubleRow rhs, off the busy
                    # engines: GPSIMD reads the finished bf16 strip. Two
                    # halves so downstream chains start at half-done.
                    nc.gpsimd.tensor_copy(dst8[:, 0:4, col8:col8 + 128],
                                          dst_T[:, 0:4, col:col + 128])
                    nc.gpsimd.tensor_copy(dst8[:, 4:8, col8:col8 + 128],
                                          dst_T[:, 4:8, col:col + 128])

            # ---------- scope B: qT/kT/v (strict stack nesting) ----------
            xmid = es.enter_context(tc.tile_pool(name="xmidp", bufs=1)).tile(
                [128, NS, E], bf16)
            # LN2 stats+apply run during the attention tail (DVE/GPSIMD);
            # only the transposes+copies stay in scope D. lc2 holds the
            # normalized (pre-transpose) chunks across the scope boundary.
            lc2 = es.enter_context(tc.tile_pool(name="ln2s", bufs=1)).tile(
                [128, NS, E], bf16)
            statp2o = es.enter_context(tc.tile_pool(name="statp2o", bufs=6))
            with ExitStack() as sB:
                qT = sB.enter_context(tc.tile_pool(name="qTp", bufs=1)).tile(
                    [128, NP, M], bf16)
                kT = sB.enter_context(tc.tile_pool(name="kTp", bufs=1)).tile(
                    [128, NP, T], bf16)
                # per-pair AV stationary operand [V_h0 | ones(64) | V_h1]:
                # head0 reads cols 0:128, head1 cols 64:192 — the shared ones
                # block makes the same matmul that accumulates attn@V also
                # accumulate the softmax denominator (replicated on the 64
                # out-rows opposite each head's data rows).
                # vS carries 32*v (fp8 wv split is stored x32); the ones
                # block becomes 32.0 so the denominator picks up the same
                # scale and do_norm's divide cancels it exactly.
                vS = sB.enter_context(tc.tile_pool(name="vp", bufs=1)).tile(
                    [128, TK, NP, 192], bf16)

                # ---------- scope A: LN1 + QKV projections ----------
                with ExitStack() as sA:
                    wpool = sA.enter_context(tc.tile_pool(name="wpool", bufs=1))
                    stage = sA.enter_context(tc.tile_pool(name="xstage", bufs=4))
                    lnstage = sA.enter_context(tc.tile_pool(name="lnstage", bufs=2))
                    statp = sA.enter_context(tc.tile_pool(name="statp", bufs=6))
                    tpsum = sA.enter_context(
                        tc.tile_pool(name="tpsum", bufs=4, space="PSUM"))
                    qps = sA.enter_context(
                        tc.tile_pool(name="qps", bufs=2, space="PSUM"))

                    # lnf chunks rotate through a 3-deep stage: V consumes
                    # each chunk as soon as it lands; Q/K read the persistent
                    # fp8 shadow lnf8.
                    lnfp = sA.enter_context(tc.tile_pool(name="lnfp", bufs=4))
                    lnf8 = sA.enter_context(tc.tile_pool(name="lnf8p", bufs=1)).tile(
                        [128, ET, T], f8)
                    # LN1 over x_full (host-permuted: q rows are chunks 0..7,
                    # the pair-core's rows are chunks 8..15) -> lnf. Q^T, each
                    # K^T token-quarter and each V chunk are emitted as soon
                    # as the lnf region they read is complete, so the PE
                    # works through projections while the LN chains run.
                    wq_sb = wpool.tile([128, ET, H * D], f8, tag="wq")
                    wk_sb = wpool.tile([128, ET, H * D], f8, tag="wk")
                    wvu_sb = wpool.tile([128, ET, H * D], f8, tag="wvu")
                    wvv_sb = wpool.tile([128, ET, H * D], f8, tag="wvv")
                    nc.sync.dma_start(wvu_sb[:], wvu.rearrange(
                        "p (et n) -> p et n", et=ET))
                    nc.sync.dma_start(wvv_sb[:], wvv.rearrange(
                        "p (et n) -> p et n", et=ET))
                    lnbp = sA.enter_context(tc.tile_pool(name="lnbp", bufs=3))

                    def v_chunk(t, lnfc):
                        # split-fp8: v = a@u + b@u + a@v with a=fp8(lnf),
                        # b=fp8(lnf-a), u/v the fp8 split of 32*wv. 12 DR
                        # matmuls replace 16 bf16 ones (0.75x PE).
                        a8 = lnf8[:, :, t * 128:(t + 1) * 128]
                        b8 = lnbp.tile([128, ET, 128], f8, tag="lnb")
                        nc.vector.tensor_tensor(b8[:, 0:4], lnfc[:, 0:4],
                                                a8[:, 0:4], OP.subtract)
                        nc.gpsimd.tensor_tensor(b8[:, 4:8], lnfc[:, 4:8],
                                                a8[:, 4:8], OP.subtract)
                        ps = qps.tile([128, 1024], f32)
                        chains = [(a8, wvu_sb), (a8, wvv_sb), (b8, wvu_sb)]
                        for ci, (act, wt) in enumerate(chains):
                            for hc in range(2):
                                cols = slice(hc * 512, (hc + 1) * 512)
                                for ep in range(ET // 2):
                                    nc.tensor.matmul(
                                        ps[:, cols],
                                        lhsT=act[:, 2 * ep:2 * ep + 2, :],
                                        rhs=wt[:, 2 * ep:2 * ep + 2, cols],
                                        start=(ci == 0 and ep == 0),
                                        stop=(ci == 2 and ep == ET // 2 - 1),
                                        perf_mode=DR,
                                        skip_group_check=True)
                        psv = ps[:].rearrange("p (np h d) -> p np h d", np=NP,
                                              h=2)
                        vbv = vb[:].rearrange("p (np h d) -> p np h d", np=NP,
                                              h=2)
                        nc.vector.tensor_add(
                            vS[:, t, :, 0:64], psv[:, :, 0, :], vbv[:, :, 0, :])
                        nc.vector.tensor_add(
                            vS[:, t, :, 128:192], psv[:, :, 1, :],
                            vbv[:, :, 1, :])

                    def k_quarter(tq, ms):
                        for m in ms:
                            ps = qps.tile([128, 512], f32)
                            for ep in range(ET // 2):
                                nc.tensor.matmul(
                                    ps[:],
                                    lhsT=wk_sb[:, 2 * ep:2 * ep + 2,
                                               m * 128:(m + 1) * 128],
                                    rhs=lnf8[:, 2 * ep:2 * ep + 2,
                                             tq * 512:(tq + 1) * 512],
                                    start=(ep == 0), stop=(ep == ET // 2 - 1),
                                    perf_mode=DR)
                            # descale+bias on ACT (idle in this phase)
                            nc.scalar.activation(
                                kT[:, m, tq * 512:(tq + 1) * 512], ps[:],
                                AF.Identity, bias=kb[:, m:m + 1],
                                scale=1.0 / 32)

                    def q_proj(ms):
                        for m in ms:
                            ps = qps.tile([128, 1024], f32)
                            for qc in range(2):
                                for ep in range(ET // 2):
                                    nc.tensor.matmul(
                                        ps[:, qc * 512:(qc + 1) * 512],
                                        lhsT=wq_sb[:, 2 * ep:2 * ep + 2,
                                                   m * 128:(m + 1) * 128],
                                        rhs=lnf8[:, 2 * ep:2 * ep + 2,
                                                 qc * 512:(qc + 1) * 512],
                                        start=(ep == 0),
                                        stop=(ep == ET // 2 - 1),
                                        perf_mode=DR)
                            nc.scalar.activation(
                                qT[:, m, :], ps[:], AF.Identity,
                                bias=qb[:, m:m + 1],
                                scale=float(E) ** -0.5 / 32)

                    for c in range(TK):
                        xc = stage.tile([128, E], bf16)
                        nc.scalar.dma_start(
                            xc[:], x_full[c * 128:(c + 1) * 128, :])
                        lnfc = lnfp.tile([128, ET, 128], bf16, tag="lnfc")
                        ln_chunk(xc[:], lnfc, 0, statp, lnstage,
                                 tpsum, c, dst8=lnf8, col8=c * 128)
                        if c == 0:
                            nc.sync.dma_start(wk_sb[:], wk.rearrange(
                                "p (et n) -> p et n", et=ET))
                        if c == 1:
                            nc.sync.dma_start(wq_sb[:], wq.rearrange(
                                "p (et n) -> p et n", et=ET))
                        v_chunk(c, lnfc)
                        if c % 4 == 3:
                            k_quarter(c // 4, range(0, 3))
                        if c % 4 == 0 and c > 0:
                            k_quarter(c // 4 - 1, range(3, 6))
                        if c % 4 == 1 and c > 1:
                            k_quarter(c // 4 - 1, range(6, ET))
                        if c == TK - 1:
                            k_quarter(3, range(3, ET))
                        if c == NS - 1:
                            q_proj(range(0, 4))
                        if c == NS + 1:
                            q_proj(range(4, ET))

                # ---------- attention + output projection ----------
                # half-outer: all pairs finish q-cols [0,512) first; the
                # output projection for q-chunks 0..3 is then dripped one
                # chunk at a time between second-half pairs so the PE works
                # through proj while ACT drains the exp/normalize backlog.
                with ExitStack() as sC:
                    oT = sC.enter_context(tc.tile_pool(name="oTp", bufs=1)).tile(
                        [128, NP, M], bf16)
                    ptp = sC.enter_context(tc.tile_pool(name="ptp", bufs=5))
                    normp = sC.enter_context(tc.tile_pool(name="normp", bufs=2))
                    xqpp = sC.enter_context(tc.tile_pool(name="xqpp", bufs=1))
                    pwp = sC.enter_context(tc.tile_pool(name="pwp", bufs=1))
                    apsum = sC.enter_context(
                        tc.tile_pool(name="apsum", bufs=2, space="PSUM"))
                    spsum = sC.enter_context(
                        tc.tile_pool(name="spsum", bufs=2, space="PSUM"))

                    maskEv = maskE[:].rearrange("p (h q) -> p h q", h=2)
                    maskOv = maskO[:].rearrange("p (h q) -> p h q", h=2)

                    pw_sb = pwp.tile([128, NP, E], bf16)
                    nc.sync.dma_start(pw_sb[:], projw.rearrange(
                        "p (m e) -> p m e", m=NP))
                    nc.gpsimd.memset(vS[:, 0:8, :, 64:128], 32.0)
                    nc.gpsimd.memset(vS[:, 8:16, :, 64:128], 32.0)
                    xq_t = {}

                    def do_norm(av, p, half):
                        # den sits on the 64 rows opposite each head's data;
                        # 1/den via the fast DVE Newton-Raphson reciprocal
                        # (dens are sums of exps, well inside its safe range)
                        # keeps the whole normalize off the exp-saturated ACT.
                        # one full-tile fast reciprocal (custom DVE ops
                        # require partition base 0; the data-row lanes are
                        # junk and never read)
                        rcp = normp.tile([128, 2, 512], f32, tag="rcp")
                        nc.vector.reciprocal_approx_fast(rcp[:], av[:])
                        colr = slice(512 * half, 512 * (half + 1))
                        for h in range(2):
                            dn = slice(64 * (1 - h), 64 * (1 - h) + 64)
                            nc.vector.tensor_mul(
                                oT[64 * h:64 * h + 64, p, colr],
                                av[64 * h:64 * h + 64, h, :],
                                rcp[dn, h, :])

                    # permuted key order: chunks 0..7 are this core's parity
                    # (incl. the causal diagonal), 8..15 the pair-core's
                    # (strictly past or future, selected by the all-ones/
                    # all-zero parity mask).
                    CHUNKS = [
                        [x for pr in zip(
                            [(kt, 128 * kt, "tri") for kt in range(4)],
                            [(kt, 128 * (kt - 8), "par")
                             for kt in range(8, 12)]) for x in pr],
                        [(kt, 0, None) for kt in range(4)] +
                        [(kt, 128 * (kt - 4), "tri") for kt in range(4, 8)] +
                        [(kt, 0, None) for kt in range(8, 12)] +
                        [(kt, 128 * (kt - 12), "par") for kt in range(12, 16)],
                    ]

                    def do_av(item):
                        av, p, half, kt, qlo, sp, pt = item
                        for h in range(2):
                            nc.tensor.matmul(
                                av[:, h, qlo:512],
                                lhsT=vS[:, kt, p, 64 * h:64 * h + 128],
                                rhs=pt[:, h, qlo:512],
                                start=(kt == 0), stop=sp,
                                skip_group_check=True)
                        if sp:
                            do_norm(av, p, half)

                    def proj_qm(qm):
                        if qm // 4 not in xq_t:
                            xqh = xqpp.tile([128, 4, E], bf16, tag="xq")
                            xq_t[qm // 4] = xqh
                        xqh = xq_t[qm // 4]
                        nc.sync.dma_start(
                            xqh[:, qm % 4, :], xqp[qm * 128:(qm + 1) * 128, :])
                        ps = apsum.tile([128, 1024], f32, tag="av")
                        for ec in range(2):
                            for pk in range(NP):
                                nc.tensor.matmul(
                                    ps[:, ec * 512:(ec + 1) * 512],
                                    lhsT=oT[:, pk, qm * 128:(qm + 1) * 128],
                                    rhs=pw_sb[:, pk, ec * 512:(ec + 1) * 512],
                                    start=(pk == 0), stop=(pk == NP - 1))
                        nc.vector.tensor_add(
                            xmid[:, qm, :], ps[:], xqh[:, qm % 4, :])

                    def ln2_sa(qm):
                        ag, rstd = ln_stats(xmid[:, qm, :], statp2o)
                        ln_apply(lc2[:, qm, :], xmid[:, qm, :], ag, rstd,
                                 statp2o, False)
                        # after LN2 consumed xmid, fold the final bf2 bias in
                        nc.gpsimd.tensor_add(xmid[:, qm, :], xmid[:, qm, :],
                                             bf2[:])

                    def attn_half(half, interleave=()):
                        chunks = CHUNKS[half]
                        last_kt = chunks[-1][0]
                        pend = []
                        for p in range(NP):
                            av = apsum.tile([128, 2, 512], f32, tag="av")
                            for kt, qlo, mk in chunks:
                                ps = spsum.tile([128, 2, 512], f32)
                                for h in range(2):
                                    nc.tensor.matmul(
                                        ps[:, h, qlo:512],
                                        lhsT=kT[64 * h:64 * h + 64, p,
                                                kt * 128:(kt + 1) * 128],
                                        rhs=qT[64 * h:64 * h + 64, p,
                                               512 * half + qlo:
                                               512 * (half + 1)],
                                        start=True, stop=True)
                                pt = ptp.tile([128, 2, 512], bf16)
                                nc.scalar.activation(
                                    pt[:, :, qlo:512], ps[:, :, qlo:512],
                                    AF.Exp)
                                if mk is not None:
                                    mkv = maskEv if mk == "tri" else maskOv
                                    nc.vector.tensor_mul(
                                        pt[:, :, qlo:qlo + 128],
                                        pt[:, :, qlo:qlo + 128], mkv)
                                pend.append((av, p, half, kt, qlo,
                                             kt == last_kt, pt))
                                if len(pend) > 4:
                                    do_av(pend.pop(0))
                            if p >= 1 and p - 1 < len(interleave):
                                while len(pend) > 4:
                                    do_av(pend.pop(0))
                                interleave[p - 1]()
                        while pend:
                            do_av(pend.pop(0))

                    attn_half(0)
                    tasks = [lambda: proj_qm(0), lambda: proj_qm(1),
                             lambda: ln2_sa(0), lambda: proj_qm(2),
                             lambda: ln2_sa(1), lambda: proj_qm(3),
                             lambda: ln2_sa(2)]
                    attn_half(1, interleave=tasks)
                    ln2_sa(3)
                    for qm in range(4, NS):
                        proj_qm(qm)
                        ln2_sa(qm)

            # ---------- scope D: LN2 + FFN ----------
            # LN2 q-chunks 0..3 interleave with the tail projections; FFN1's
            # first-half fm groups interleave with LN2 chunks 4..7 so the
            # PE never drains while DVE/ACT run the layernorm chains.
            with ExitStack() as sD:
                ln2p = sD.enter_context(tc.tile_pool(name="ln2p", bufs=1))
                ln2a = ln2p.tile([128, ET, M], f8, tag="a")
                ln2b = ln2p.tile([128, ET, M], f8, tag="b")
                w2p = sD.enter_context(tc.tile_pool(name="w2p", bufs=1))
                w2u_sb = w2p.tile([128, FT, E], f8, tag="u")
                w2v_sb = w2p.tile([128, FT, E], f8, tag="v")

                tpsum2 = sD.enter_context(
                    tc.tile_pool(name="tpsum2", bufs=2, space="PSUM"))
                rtp = sD.enter_context(tc.tile_pool(name="rtp", bufs=1))
                rbfp = sD.enter_context(tc.tile_pool(name="rbfp", bufs=3))
                w1p = sD.enter_context(tc.tile_pool(name="w1p", bufs=6))
                zps = sD.enter_context(
                    tc.tile_pool(name="zps", bufs=2, space="PSUM"))
                ops = sD.enter_context(
                    tc.tile_pool(name="ops", bufs=2, space="PSUM"))
                outp = sD.enter_context(tc.tile_pool(name="outp", bufs=3))

                def ln2_qm(qm):
                    # stats+apply already ran in the attention tail. Here:
                    # transpose (PE), then the fp8 split stores — a on ACT,
                    # b = tp - a on DVE.
                    col = qm * 128
                    lc = lc2[:, qm, :]
                    for et in range(ET):
                        tp = tpsum2.tile([128, 128], bf16)
                        nc.tensor.transpose(tp[:],
                                            lc[:, et * 128:(et + 1) * 128],
                                            ident[:])
                        nc.scalar.copy(ln2a[:, et, col:col + 128], tp[:])
                        nc.vector.tensor_tensor(
                            ln2b[:, et, col:col + 128], tp[:],
                            ln2a[:, et, col:col + 128], OP.subtract)

                rT_tiles = {}

                def ffn1_fm(half, fm):
                    if half not in rT_tiles:
                        ra = rtp.tile([128, FT, 512], f8, tag="rTa")
                        rb = rtp.tile([128, FT, 512], f8, tag="rTb")
                        rT_tiles[half] = (ra, rb)
                    ra, rb = rT_tiles[half]
                    w1f = w1p.tile([128, 2, ET, 128], f8)
                    nc.sync.dma_start(
                        w1f[:, 0], w1u.rearrange("p (fm et f) -> p fm et f",
                                                 fm=FT, et=ET)[:, fm])
                    nc.sync.dma_start(
                        w1f[:, 1], w1v.rearrange("p (fm et f) -> p fm et f",
                                                 fm=FT, et=ET)[:, fm])
                    zp = zps.tile([128, 512], f32)
                    cols = slice(half * 512, (half + 1) * 512)
                    chains = [(ln2a, 0), (ln2a, 1), (ln2b, 0)]
                    for ci, (act, wi) in enumerate(chains):
                        for ep in range(ET // 2):
                            nc.tensor.matmul(
                                zp[:],
                                lhsT=w1f[:, wi, 2 * ep:2 * ep + 2, :],
                                rhs=act[:, 2 * ep:2 * ep + 2, cols],
                                start=(ci == 0 and ep == 0),
                                stop=(ci == 2 and ep == ET // 2 - 1),
                                perf_mode=DR)
                    # relu (scale descales the x32 of w1), then fp8 split:
                    # a copy on ACT, b = r - a on DVE.
                    # rbf = relu(z + 32*b1) = 32*relu(z/32+b1); the x32
                    # rides the residual stream (host-descaled x2048).
                    rbf = rbfp.tile([128, 512], bf16)
                    nc.scalar.activation(rbf[:], zp[:], AF.Relu,
                                         bias=b1[:, fm:fm + 1])
                    # fp8 split: a on the idle GPSIMD (SBUF->SBUF), b on DVE
                    nc.gpsimd.tensor_copy(ra[:, fm, :], rbf[:])
                    nc.vector.tensor_tensor(rb[:, fm, :], rbf[:],
                                            ra[:, fm, :], OP.subtract)

                def ffn2_qq(half, qq):
                    ra, rb = rT_tiles[half]
                    qm = half * 4 + qq
                    ot = outp.tile([128, E], f32)
                    op = ops.tile([128, 1024], f32)
                    for ec in range(2):
                        cols = slice(ec * 512, (ec + 1) * 512)
                        chains = [(ra, w2u_sb), (ra, w2v_sb), (rb, w2u_sb)]
                        for ci, (act, wt) in enumerate(chains):
                            for fp_ in range(FT // 2):
                                nc.tensor.matmul(
                                    op[:, cols],
                                    lhsT=act[:, 2 * fp_:2 * fp_ + 2,
                                             qq * 128:(qq + 1) * 128],
                                    rhs=wt[:, 2 * fp_:2 * fp_ + 2, cols],
                                    start=(ci == 0 and fp_ == 0),
                                    stop=(ci == 2 and fp_ == FT // 2 - 1),
                                    perf_mode=DR)
                        nc.vector.tensor_add(
                            ot[:, cols], op[:, cols], xmid[:, qm, cols])
                        nc.scalar.dma_start(
                            out[qm * 128:(qm + 1) * 128, cols], ot[:, cols])

                for qm in range(4):
                    ln2_qm(qm)
                # LN2 chunks 4..7 drip between the first 4 fm-groups of
                # FFN1-half0 (which only needs ln2T token cols 0:512).
                for g in range(4):
                    ln2_qm(4 + g)
                    nc.sync.dma_start(
                        w2u_sb[:, g * 8:(g + 1) * 8, :],
                        w2u.rearrange("p (ft e) -> p ft e",
                                      ft=FT)[:, g * 8:(g + 1) * 8, :])
                    nc.sync.dma_start(
                        w2v_sb[:, g * 8:(g + 1) * 8, :],
                        w2v.rearrange("p (ft e) -> p ft e",
                                      ft=FT)[:, g * 8:(g + 1) * 8, :])
                    for fm in range(g * 8, g * 8 + 8):
                        ffn1_fm(0, fm)
                for qq in range(4):
                    ffn2_qq(0, qq)
                rT_tiles.pop(0)
                for fm in range(FT):
                    ffn1_fm(1, fm)
                for qq in range(4):
                    ffn2_qq(1, qq)

    nc.compile()
    _CACHE[key] = nc
    return nc


def _prep_inputs(x, wq, wk, wv, proj_w, proj_b, g1, beta1, g2, beta2, w1, bf1,
                 w2, bf2):
    """Host-side sharding + weight folding. Returns list of 8 in_maps."""
    f32 = np.float32
    x = np.asarray(x, f32)
    scale = float(E) ** -0.5

    Wq = np.asarray(wq, f32).transpose(1, 0, 2).reshape(E, H * D)
    Wk = np.asarray(wk, f32).transpose(1, 0, 2).reshape(E, H * D)
    Wv = np.asarray(wv, f32).transpose(1, 0, 2).reshape(E, H * D)
    g1 = np.asarray(g1, f32)
    beta1 = np.asarray(beta1, f32)
    g2 = np.asarray(g2, f32)
    beta2 = np.asarray(beta2, f32)
    w1 = np.asarray(w1, f32)
    w2 = np.asarray(w2, f32)
    bf1 = np.asarray(bf1, f32)
    bf2 = np.asarray(bf2, f32)
    proj_w = np.asarray(proj_w, f32)
    proj_b = np.asarray(proj_b, f32)

    def sb_layout(w, ntile):
        # [ntile*128, N] -> [128, ntile*N] with per-partition contiguous tiles
        n = w.shape[1]
        return np.ascontiguousarray(
            w.reshape(ntile, 128, n).transpose(1, 0, 2).reshape(128, ntile * n))

    # fp8 weight scaling: x32 (x64 for w2) puts sigma at ~1 inside e4m3's
    # normal range. V/FFN weights are double-split (u = fp8(w*s),
    # v = fp8(w*s - u)) for the 3-chain split matmuls; Q/K use u only.
    def split8(w, s):
        u = (w * s).astype(F8)
        v = ((w * s) - u.astype(f32)).astype(F8)
        return u, v

    wq_b = sb_layout((Wq * g1[:, None] * 32).astype(F8), ET)
    wk_b = sb_layout((Wk * g1[:, None] * 32).astype(F8), ET)
    wvu_n, wvv_n = split8(Wv * g1[:, None], 32)
    wvu_b, wvv_b = sb_layout(wvu_n, ET), sb_layout(wvv_n, ET)
    qbias = (beta1 @ Wq) * scale
    kbias = beta1 @ Wk
    vbias = (beta1 @ Wv) * 32
    w1u_n, w1v_n = split8(w1 * g2[:, None], 32)

    def w1_layout(w):
        return np.ascontiguousarray(
            w.reshape(ET, 128, FT, 128).transpose(1, 2, 0, 3)
            .reshape(128, FT * ET * 128))

    w1u_b, w1v_b = w1_layout(w1u_n), w1_layout(w1v_n)
    b1v = bf1 + beta2 @ w1
    w2u_n, w2v_n = split8(w2, 64)
    w2u_b, w2v_b = sb_layout(w2u_n, FT), sb_layout(w2v_n, FT)
    # the residual stream runs x64 on-chip (w2 split carries it; proj_w and
    # xqp are pre-scaled to match); divided back out on the host.
    projw_b = sb_layout((proj_w * 2048).astype(BF16), NP)

    qb = np.ascontiguousarray(qbias.reshape(ET, 128).T, f32)
    kb = np.ascontiguousarray(kbias.reshape(ET, 128).T, f32)
    vb = np.ascontiguousarray(np.broadcast_to(vbias, (128, H * D))).astype(BF16)
    b1m = np.ascontiguousarray(b1v.reshape(FT, 128).T * 32, f32)
    bf2m = np.ascontiguousarray(
        np.broadcast_to(bf2 * 2048, (128, E))).astype(BF16)

    tri = np.triu(np.ones((128, 128), f32))  # [k_row, q_col]: 1 iff k <= q
    zerosm = np.zeros((128, 128), f32)
    # maskE = causal diagonal (all cores); maskO = parity: the pair-core's
    # diagonal-adjacent chunk is strictly past (odd cores) or future (even).
    mO = {0: zerosm, 1: tri * 0 + 1}
    in_maps = []
    for c in range(NCORES):
        b, hpar = c // 2, c % 2
        xc = x[b].reshape(TK, 128, E)
        xq = np.ascontiguousarray(xc[hpar::2].reshape(M, E), f32)
        xperm = np.ascontiguousarray(
            np.concatenate([xc[hpar::2], xc[1 - hpar::2]], axis=0)
            .reshape(T, E)).astype(BF16)
        in_maps.append({
            "x_full": xperm,
            "xqp": ((xq + proj_b[None, :].astype(f32)) * 2048).astype(BF16),
            "wq": wq_b, "wk": wk_b, "wvu": wvu_b, "wvv": wvv_b,
            "projw": projw_b, "w1u": w1u_b, "w1v": w1v_b,
            "w2u": w2u_b, "w2v": w2v_b,
            "qb": qb, "kb": kb, "vb": vb, "b1": b1m, "bf2b": bf2m,
            "maskE": np.ascontiguousarray(
                np.tile(tri, (1, 2))).astype(BF16),
            "maskO": np.ascontiguousarray(
                np.tile(mO[hpar], (1, 2))).astype(BF16),
        })
    return in_maps


def _run(inputs, trace=False):
    from concourse.bass_utils import run_bass_kernel_spmd
    nc = _build()
    in_maps = _prep_inputs(**inputs)
    res = run_bass_kernel_spmd(nc, in_maps, core_ids=list(range(NCORES)),
                               trace=trace)
    full = np.empty((B, T, E), np.float32)
    for c in range(NCORES):
        b, hpar = c // 2, c % 2
        full[b].reshape(TK, 128, E)[hpar::2] = (
            res.results[c]["out"].reshape(NS, 128, E) * np.float32(1 / 2048))
    return full, res


def kernel(**inputs) -> np.ndarray:
    out, _ = _run(inputs, trace=False)
    return out

